# revision 5
# baseline (speedup 1.0000x reference)
"""Trainium2 Bass kernel for causal multi-head attention (B=4, T=2048, D=1024, H=16).

Sharding: tensor-parallel over heads for QKV+attention (each of 8 cores owns
2 heads over all tokens), then AllToAll re-shards from head-sharded to
token-sharded so each core computes the final FC over the full feature dim
for its 256-token slice of each batch.

All matmuls run in bf16 (fp32 streams at half PE rate; bf16 at full), with
fp32 PSUM accumulation. Scores are computed transposed (S^T = K Q^T, two
heads packed in PE quadrants via tile_position) so softmax normalization
lands on the PV matmul's free dim; the denominator comes from a ones column
augmented into V. Normalization is deferred out of the inner loop (the
per-chunk broadcast chain caused PE bubbles + HAM re-throttling). The
denominator reciprocal is computed as exp(-ln(x)) on the scalar engine: Ln
and Exp share one activation table set (natural_log_exp_and_others), so the
scalar engine never reloads tables mid-kernel (the old Reciprocal function
lives in its own set and caused 32 x 1.3us ACT_TABLE_LOAD stalls on the
exp critical path). QKV/FC biases are added on the DVE during the
PSUM->SBUF copy (tensor_scalar with a per-partition [128,1] bias AP, or a
precomputed broadcast tile for V whose bias varies along the free dim) --
the old ones-vector bias matmuls burned ~27us of PE streaming+LDWEIGHTS.
The broadcast of 1/denom across partitions uses a selector matmul
(partition-shifted DVE/DMA ops misbehave; SBUF-side DMA access patterns
must keep the partition dim outermost). Mask-multiplies and normalize
multiplies run on the otherwise-idle GPSIMD engine (it cannot read PSUM);
diagonal-tile exp+mask handle both heads in one instruction via strided 3D
APs (mask tile duplicated per head). Normalization runs per query-chunk
(one chunk behind attention) and stages its praw slice into the AllToAll
DRAM buffer immediately, so the collective trigger's DMA wait is nearly
zero and the next batch's gpsimd mask-muls are not stalled behind it.
One AllToAll per batch (smaller per-half collectives measured ~3x slower
per byte), QKV of batch b+1 / FC of batch b-1 interleave into batch b's
attention to keep the PE fed while the scalar engine runs exp, and FC of
batch 2 is deferred into the tail to overlap the final AllToAll.
"""
import sys

for _p in ("/opt/trn_rl_repo",):
    if _p not in sys.path:
        sys.path.insert(0, _p)

import numpy as np

import concourse.bass as bass
import concourse.mybir as mybir
import concourse.tile as tile
from concourse import bacc
from concourse.bass_utils import run_bass_kernel_spmd

f32 = mybir.dt.float32
bf16 = mybir.dt.bfloat16
EXP = mybir.ActivationFunctionType.Exp
LN = mybir.ActivationFunctionType.Ln

B, T, D, H, HD = 4, 2048, 1024, 16, 64
NCORES = 8
HPC = H // NCORES          # heads per core = 2
BT = B * T                 # 8192
CH = 512                   # token chunk (q chunk / projection chunk)
NCH_B = T // CH            # 4 projection chunks per batch
QC = T // CH               # 4 query chunks per batch
NKV_B = T // 128           # 16 kv tiles of 128 per batch
TOK = 256                  # tokens per core per batch (after AllToAll)
SCALE = 1.0 / 8.0          # 1/sqrt(HD)

_CACHE = {}


class _Bacc(bacc.Bacc):
    """Bacc whose activation-table pass resolves Exp AND Ln to the combined
    natural_log_exp_and_others set (canonical id 6) instead of their
    individual default sets, so alternating Exp/Ln activations trigger one
    ACT_TABLE_LOAD total instead of one per transition (1.3us each, on the
    softmax critical path)."""

    def insert_act_table_loads(self):
        has_activation = any(
            isinstance(i, mybir.InstActivation)
            for b in self.main_func.blocks
            for i in b.instructions
        )
        if not has_activation:
            return
        from concourse.hw_specs import get_activation_tables
        tables = []
        for name, funcs in get_activation_tables(self.m.arch).items():
            funcs = set(funcs)
            if name == "exp_and_others":
                funcs.discard(mybir.ActivationFunctionType.Exp)
            if name == "natural_log":
                funcs.discard(mybir.ActivationFunctionType.Ln)
            tables.append((name, funcs))
        bacc._bass_rust.insert_act_table_loads(self, tables)


def _build(no_collective=False):
    nc = _Bacc("TRN2", target_bir_lowering=False, debug=False,
               num_devices=NCORES)

    xT = nc.dram_tensor("xT", [D, BT], bf16, kind="ExternalInput").ap()
    wq_d = nc.dram_tensor("wq", [128, 8 * 384], bf16, kind="ExternalInput").ap()
    bqc_d = nc.dram_tensor("bqc", [128, 2], f32, kind="ExternalInput").ap()
    vbb_d = nc.dram_tensor("vbb", [128, 128], bf16, kind="ExternalInput").ap()
    wfc_d = nc.dram_tensor("wfc", [128, 64 * 128], bf16,
                           kind="ExternalInput").ap()
    bfcc_d = nc.dram_tensor("bfcc", [128, 8], f32, kind="ExternalInput").ap()
    hm_d = nc.dram_tensor("hm", [128, 2 * 896], bf16, kind="ExternalInput").ap()
    outT = nc.dram_tensor("outT", [D, B * TOK], f32,
                          kind="ExternalOutput").ap()

    with tile.TileContext(nc) as tc:
        with tc.tile_pool(name="const", bufs=1) as cst, \
             tc.tile_pool(name="dram", bufs=1, space="DRAM") as dpool, \
             tc.tile_pool(name="work", bufs=1) as wk, \
             tc.tile_pool(name="ps", bufs=1, space="PSUM") as ps:

            # ---- weights needed first (QKV of batch 0) ----
            wq = cst.tile([128, 8 * 384], bf16)
            nc.sync.dma_start(wq[:], wq_d[:])
            bqc = cst.tile([128, 2], f32)
            nc.sync.dma_start(bqc[:], bqc_d[:])
            vbb = cst.tile([128, 128], bf16)
            nc.sync.dma_start(vbb[:], vbb_d[:])
            onesb = cst.tile([1, CH], bf16)
            nc.gpsimd.memset(onesb[:], 1.0)
            # selector for reciprocal broadcast: row 64 = 1, rest 0
            zl = cst.tile([65, 64], bf16)
            nc.gpsimd.memset(zl[:], 0.0)
            nc.gpsimd.memset(zl[64:65, :], 1.0)

            # ---- deferred constants (needed later; don't block first mm) ----
            def _late_consts():
                hm = cst.tile([128, 2 * 896], bf16)
                nc.sync.dma_start(hm[:], hm_d[:])
                wfc = cst.tile([128, 64 * 128], bf16)
                nc.sync.dma_start(wfc[:], wfc_d[:])
                bfcc = cst.tile([128, 8], f32)
                nc.sync.dma_start(bfcc[:], bfcc_d[:])
                # per-head reciprocal staging: row 64 = recip, rows 0-63
                # zeroed once (garbage would poison the selector matmul)
                rc = []
                for h in range(HPC):
                    t = cst.tile([65, T], bf16, name=f"rc{h}")
                    nc.gpsimd.memset(t[0:64, :], 0.0)
                    rc.append(t)
                return hm, wfc, bfcc, rc

            # ---- per-batch state (double buffered across the pipeline) ----
            def alloc_batch(b):
                qt = wk.tile([128, T], bf16, tag="qt", bufs=2, name=f"qt{b}")
                kt = wk.tile([128, T], bf16, tag="kt", bufs=2, name=f"kt{b}")
                vsb = wk.tile([128, NKV_B * 130], bf16, tag="vsb", bufs=2,
                              name=f"vsb{b}")
                v3 = vsb.rearrange("p (t c) -> p t c", c=130)
                nc.gpsimd.memset(v3[:, :, 64:65], 1.0)
                nc.gpsimd.memset(v3[:, :, 129:130], 1.0)
                praw = [wk.tile([64, T], bf16, tag=f"praw{h}", bufs=2,
                                name=f"praw{h}_{b}") for h in range(HPC)]
                return qt, kt, vsb, praw

            def qkv_dma(b, ch):
                """Prefetch one 512-token x chunk."""
                c0 = b * T + ch * CH
                xt = wk.tile([128, 8 * CH], bf16, tag="xt", bufs=3,
                             name=f"xt{b}_{ch}")
                xt3 = xt.rearrange("p (d c) -> p d c", d=8)
                xs3 = xT[:, c0:c0 + CH].rearrange("(d p) c -> p d c", p=128)
                nc.sync.dma_start(xt3[:], xs3)
                return xt

            def qkv_units(b, ch, xt, st):
                """Chunk projection as self-contained PE units (aux PSUM).

                Each unit allocates its own short-lived [128,512] aux tile so
                units can interleave with attention without pinning the
                st-tag rotation."""
                qt, kt, vsb = st[0], st[1], st[2]
                cs = ch * CH
                xt3 = xt.rearrange("p (d c) -> p d c", d=8)

                def q_unit(tgt, wo, bcol):
                    def emit():
                        pq = ps.tile([128, CH], f32, tag="aux", bufs=2,
                                     name=f"pq{b}_{ch}_{wo}")
                        for d in range(8):
                            nc.tensor.matmul(pq[:],
                                             wq[:, d * 384 + wo:d * 384 + wo + 128],
                                             xt[:, d * CH:(d + 1) * CH],
                                             start=(d == 0), stop=(d == 7))
                        nc.vector.tensor_scalar_add(
                            tgt[:, cs:cs + CH], pq[:], bqc[:, bcol:bcol + 1])
                    return emit

                def v_unit(sb):
                    def emit():
                        kvt = ch * 4 + sb
                        pvv = ps.tile([128, CH], f32, tag="aux", bufs=2,
                                      name=f"pvv{b}_{ch}_{sb}")
                        for d in range(8):
                            nc.tensor.matmul(
                                pvv[:, 0:128],
                                xt3[:, d, sb * 128:(sb + 1) * 128],
                                wq[:, d * 384 + 256:d * 384 + 384],
                                start=(d == 0), stop=(d == 7))
                        base = kvt * 130
                        nc.vector.tensor_add(vsb[:, base:base + 64],
                                             pvv[:, 0:64], vbb[:, 0:64])
                        nc.vector.tensor_add(vsb[:, base + 65:base + 129],
                                             pvv[:, 64:128], vbb[:, 64:128])
                    return emit

                return [q_unit(qt, 0, 0), q_unit(kt, 128, 1),
                        v_unit(0), v_unit(1), v_unit(2), v_unit(3)]

            def attn_qc(b, qc, st, hm, rc, drain):
                """Causal attention for query chunk qc of batch b."""
                qt, kt, vsb, praw = st
                nkv = 4 * (qc + 1)
                pv = [ps.tile([65, CH], f32, tag=f"pv{h}", bufs=1,
                              name=f"pv{h}_{b}_{qc}") for h in range(HPC)]
                hm3 = hm.rearrange("p (h c) -> p h c", h=2)
                for ki in range(nkv):
                    diag = ki - 4 * qc
                    # on diagonal tiles, queries below the block are fully
                    # masked: shrink the moving dim to the causal range
                    off = 128 * diag if diag > 0 else 0
                    m = CH - off
                    stt = ps.tile([128, 2 * CH], f32, tag="st", bufs=2,
                                  name=f"s_{b}_{qc}_{ki}")
                    pt = wk.tile([128, 2 * CH], bf16, tag="pt", bufs=3,
                                 name=f"p_{b}_{qc}_{ki}")
                    for h in range(HPC):
                        nc.tensor.matmul(
                            stt[:, h * CH + off:(h + 1) * CH],
                            kt[64 * h:64 * h + 64, ki * 128:(ki + 1) * 128],
                            qt[64 * h:64 * h + 64,
                               qc * CH + off:(qc + 1) * CH],
                            start=True, stop=True,
                            tile_position=(64 * h, 0))
                    # fill the scores->exp->PV latency with independent PE
                    # work: the unit lands between S(ki) and PV(ki) in the
                    # in-order PE queue, so PV no longer stalls on exp
                    drain(1)
                    if diag > 0:
                        st3 = stt.rearrange("p (h c) -> p h c", h=2)
                        pt3 = pt.rearrange("p (h c) -> p h c", h=2)
                        nc.scalar.activation(pt3[:, :, off:CH],
                                             st3[:, :, off:CH],
                                             EXP, scale=SCALE)
                    else:
                        nc.scalar.activation(pt[:], stt[:], EXP, scale=SCALE)
                    if diag >= 0:
                        pt3 = pt.rearrange("p (h c) -> p h c", h=2)
                        nc.gpsimd.tensor_mul(pt3[:, :, off:CH],
                                             pt3[:, :, off:CH],
                                             hm3[:, :, 384:384 + m])
                    for h in range(HPC):
                        vb = ki * 130 + 65 * h
                        nc.tensor.matmul(pv[h][0:65, off:CH],
                                         vsb[:, vb:vb + 65],
                                         pt[:, h * CH + off:(h + 1) * CH],
                                         start=(ki == 0),
                                         stop=(ki == nkv - 1))
                # stash raw PV + 1/denominator = exp(-ln(denom)); Ln+Exp
                # share one table set so no ACT_TABLE_LOAD is triggered.
                # ln intermediate kept f32 (bf16 would cost ~2% in exp).
                for h in range(HPC):
                    nc.vector.tensor_copy(praw[h][:, qc * CH:(qc + 1) * CH],
                                          pv[h][0:64, :])
                    lnt = wk.tile([1, CH], f32, tag="lnt", bufs=2,
                                  name=f"lnt{h}_{b}_{qc}")
                    nc.scalar.activation(lnt[:], pv[h][64:65, :], LN)
                    nc.scalar.activation(rc[h][64:65, qc * CH:(qc + 1) * CH],
                                         lnt[:], EXP, scale=-1.0)

            def normalize_qc(b, qc, st, rc, ag_in):
                """praw[:, qc] *= broadcast(1/denom), then stage into the
                AllToAll input so the collective trigger has ~no DMA wait."""
                praw = st[3]
                div = ag_in.rearrange("(d p) c -> p d c", p=128)
                for h in range(HPC):
                    bcq = ps.tile([128, CH], f32, tag="aux", bufs=2,
                                  name=f"bc{h}_{b}_{qc}")
                    nc.tensor.matmul(bcq[0:64, :], zl[:, 0:64],
                                     rc[h][:, qc * CH:(qc + 1) * CH],
                                     start=True, stop=True)
                    rbs = wk.tile([64, CH], bf16, tag="rbs", bufs=2,
                                  name=f"rbs{h}_{b}_{qc}")
                    nc.vector.tensor_copy(rbs[:], bcq[0:64, :])
                    nc.gpsimd.tensor_mul(
                        praw[h][:, qc * CH:(qc + 1) * CH],
                        praw[h][:, qc * CH:(qc + 1) * CH], rbs[:])
                    src = praw[h][:, qc * CH:(qc + 1) * CH].rearrange(
                        "p (d c) -> p d c", c=TOK)
                    nc.sync.dma_start(
                        div[64 * h:64 * h + 64, 2 * qc:2 * qc + 2, :], src)

            def a2a_alloc(b):
                ag_in = dpool.tile([NCORES * 128, TOK], bf16,
                                   name=f"ag_in{b}")
                ag_out = dpool.tile([NCORES * 128, TOK], bf16,
                                    name=f"ag_out{b}")
                return ag_in, ag_out

            def a2a_batch(b, ag_in, ag_out):
                """Re-shard batch b attention output: head- to token-sharded."""
                if no_collective:
                    nc.sync.dma_start(ag_out[:], ag_in[:])
                else:
                    nc.gpsimd.collective_compute(
                        "AllToAll", mybir.AluOpType.bypass,
                        replica_groups=[list(range(NCORES))],
                        ins=[ag_in.opt()], outs=[ag_out.opt()])
                return ag_out

            def fc_units(b, ag_out, wfc, bfcc):
                """Full FC for this core's 256-token slice, as PE units."""
                box = {}

                def load():
                    fci = wk.tile([128, 8 * TOK], bf16, tag="fci", bufs=2,
                                  name=f"fci{b}")
                    fci3 = fci.rearrange("p (d c) -> p d c", d=8)
                    srcv = ag_out.rearrange("(d p) c -> p d c", p=128)
                    nc.sync.dma_start(fci3[:], srcv)
                    box["fci"] = fci
                    box["ost"] = wk.tile([128, 8 * TOK], f32, tag="ost",
                                         bufs=2, name=f"ost{b}")

                def fo_unit(fo):
                    def emit():
                        fci, ost = box["fci"], box["ost"]
                        pfc = ps.tile([128, CH], f32, tag="aux", bufs=2,
                                      name=f"pfc{b}_{fo}")
                        for d in range(8):
                            nc.tensor.matmul(
                                pfc[:, 0:TOK],
                                wfc[:, (fo * 8 + d) * 128:
                                     (fo * 8 + d + 1) * 128],
                                fci[:, d * TOK:(d + 1) * TOK],
                                start=(d == 0), stop=(d == 7))
                        nc.vector.tensor_scalar_add(
                            ost[:, fo * TOK:(fo + 1) * TOK], pfc[:, 0:TOK],
                            bfcc[:, fo:fo + 1])
                    return emit

                def store():
                    dst = outT.rearrange("(f p) c -> p f c", p=128)[
                        :, :, b * TOK:(b + 1) * TOK]
                    osrc = box["ost"].rearrange("p (f c) -> p f c", c=TOK)
                    nc.sync.dma_start(dst, osrc)

                return [load] + [fo_unit(fo) for fo in range(8)] + [store]

            def dummy_unit(tag_i):
                """~0.9us of dependency-free matmuls to keep the HAM warm."""
                def emit():
                    pdm = ps.tile([128, CH], f32, tag="aux", bufs=2,
                                  name=f"pdm{tag_i}")
                    for r in range(4):
                        nc.tensor.matmul(pdm[0:1, :], onesb[0:1, 0:1],
                                         onesb[0:1, :],
                                         start=True, stop=True)
                return emit

            # ================= schedule =================
            from collections import deque
            filler = deque()

            def drain(n):
                for _ in range(min(n, len(filler))):
                    filler.popleft()()

            def drain_all():
                while filler:
                    filler.popleft()()

            states = [None] * B
            states[0] = alloc_batch(0)
            xts = {(0, 0): qkv_dma(0, 0)}
            hm, wfc, bfcc, rc = _late_consts()
            # warm up the collective stream so the first real AllToAll does
            # not absorb the cross-core barrier + algorithm setup (~20us)
            if not no_collective:
                w_in = dpool.tile([NCORES, 16], bf16, name="warm_in")
                w_out = dpool.tile([NCORES, 16], bf16, name="warm_out")
                wsb = cst.tile([8, 16], bf16, name="warm_sb")
                nc.gpsimd.memset(wsb[:], 0.0)
                nc.sync.dma_start(w_in[:], wsb[:])
                nc.gpsimd.collective_compute(
                    "AllToAll", mybir.AluOpType.bypass,
                    replica_groups=[list(range(NCORES))],
                    ins=[w_in.opt()], outs=[w_out.opt()])
            for ch in range(NCH_B):
                if ch + 1 < NCH_B:
                    xts[(0, ch + 1)] = qkv_dma(0, ch + 1)
                for u in qkv_units(0, ch, xts.pop((0, ch)), states[0]):
                    u()

            ags = [a2a_alloc(b) for b in range(B)]
            ndum = 0
            # QKV of batch X is staggered: chunks 0/1 run as filler during
            # batch X-1 (qc2/qc3), chunks 2/3 during batch X itself
            # (qc0/qc1).  This gives EVERY batch -- including the last --
            # ~10us of real PE filler in its first half, where previously
            # batch 3 had nothing and the PE micro-idled between S and PV
            # long enough for the HAM to hold the clock at K=4/8 for the
            # whole final quarter of the kernel.
            for b in range(B):
                for qc in range(QC):
                    if b >= 1 and qc in (0, 1):
                        ch = qc + 2
                        xts[(b, ch)] = qkv_dma(b, ch)
                        filler.extend(qkv_units(b, ch, xts.pop((b, ch)),
                                                states[b]))
                    if b + 1 < B and qc in (2, 3):
                        ch = qc - 2
                        if ch == 0:
                            states[b + 1] = alloc_batch(b + 1)
                        xts[(b + 1, ch)] = qkv_dma(b + 1, ch)
                        filler.extend(qkv_units(b + 1, ch,
                                                xts.pop((b + 1, ch)),
                                                states[b + 1]))
                    if qc == 2 and b >= 1:
                        filler.extend(fc_units(b - 1, ags[b - 1][1],
                                               wfc, bfcc))
                    if b == B - 1 and qc == 3:
                        # no next-batch QKV to interleave: keep PE warm
                        for _ in range(6):
                            ndum += 1
                            filler.append(dummy_unit(ndum))
                    attn_qc(b, qc, states[b], hm, rc, drain)
                    if qc >= 1:
                        normalize_qc(b, qc - 1, states[b], rc, ags[b][0])
                drain_all()
                normalize_qc(b, QC - 1, states[b], rc, ags[b][0])
                a2a_batch(b, *ags[b])
            # tail: dummies cover the AllToAll(3) window, then final FC
            for _ in range(6):
                ndum += 1
                dummy_unit(ndum)()
            for u in fc_units(B - 1, ags[B - 1][1], wfc, bfcc):
                u()

    nc.compile()
    return nc


def _host_inputs(x, W_qkv, b_qkv, W_fc, b_fc):
    import ml_dtypes
    bf = ml_dtypes.bfloat16
    x = np.asarray(x, dtype=np.float32)
    W_qkv = np.asarray(W_qkv, dtype=np.float32)
    b_qkv = np.asarray(b_qkv, dtype=np.float32)
    W_fc = np.asarray(W_fc, dtype=np.float32)
    b_fc = np.asarray(b_fc, dtype=np.float32)

    xT = np.ascontiguousarray(x.reshape(BT, D).T).astype(bf)
    hm1 = (np.arange(128)[:, None]
           <= np.arange(896)[None, :] - 384).astype(bf)
    hm = np.ascontiguousarray(np.concatenate([hm1, hm1], axis=1))
    # full FC weights prepacked to SBUF layout [p, (f*8+d)*128 + c]
    wfc = np.ascontiguousarray(
        W_fc.reshape(8, 128, 8, 128).transpose(1, 2, 0, 3).reshape(128, 8192)
    ).astype(bf)
    # FC bias: column fo = features fo*128..(fo+1)*128 (per-partition)
    bfcc = np.ascontiguousarray(b_fc.reshape(8, 128).T).astype(np.float32)
    in_maps = []
    for c in range(NCORES):
        f0 = c * (HPC * HD)  # 128*c
        wqs = np.concatenate(
            [W_qkv[:, p * D + f0: p * D + f0 + 128] for p in range(3)],
            axis=1)  # [1024, 384] = [q|k|v]
        wq_c = np.ascontiguousarray(
            wqs.reshape(8, 128, 384).transpose(1, 0, 2).reshape(128, 3072)
        ).astype(bf)
        # q/k bias as per-partition columns [128, 2]
        bqc_c = np.ascontiguousarray(np.stack(
            [b_qkv[f0:f0 + 128], b_qkv[D + f0:D + f0 + 128]], axis=1)
        ).astype(np.float32)
        # v bias broadcast tile [128 rows (tokens), 128 cols (features)]
        vbb_c = np.ascontiguousarray(np.broadcast_to(
            b_qkv[2 * D + f0:2 * D + f0 + 128][None, :], (128, 128))
        ).astype(bf)
        in_maps.append({
            "xT": xT, "wq": wq_c, "bqc": bqc_c, "vbb": vbb_c, "wfc": wfc,
            "bfcc": bfcc, "hm": hm,
        })
    return in_maps


def _get_nc():
    if "nc" not in _CACHE:
        _CACHE["nc"] = _build()
    return _CACHE["nc"]


def _assemble(results):
    full = np.empty((BT, D), dtype=np.float32)
    for c in range(NCORES):
        o = results[c]["outT"]  # [1024 features, 4*256 tokens]
        for b in range(B):
            full[b * T + c * TOK: b * T + (c + 1) * TOK, :] = \
                o[:, b * TOK:(b + 1) * TOK].T
    return full.reshape(B, T, D)


def kernel(x, W_qkv, b_qkv, W_fc, b_fc):
    nc = _get_nc()
    in_maps = _host_inputs(x, W_qkv, b_qkv, W_fc, b_fc)
    res = run_bass_kernel_spmd(nc, in_maps, list(range(NCORES)))
    return _assemble(res.results)


# revision 7
# speedup vs baseline: 1.1179x; 1.1179x over previous
"""Trainium2 Bass kernel for causal multi-head attention (B=4, T=2048, D=1024, H=16).

Sharding: tensor-parallel over heads for QKV+attention (each of 8 cores owns
2 heads over all tokens), then AllToAll re-shards from head-sharded to
token-sharded so each core computes the final FC over the full feature dim
for its 256-token slice of each batch.

All matmuls run in bf16 (fp32 streams at half PE rate; bf16 at full), with
fp32 PSUM accumulation. Scores are computed transposed (S^T = K Q^T, two
heads packed in PE quadrants via tile_position) so softmax normalization
lands on the PV matmul's free dim; the denominator comes from a ones column
augmented into V. Normalization is deferred out of the inner loop (the
per-chunk broadcast chain caused PE bubbles + HAM re-throttling). The
denominator reciprocal is computed as exp(-ln(x)) on the scalar engine: Ln
and Exp share one activation table set (natural_log_exp_and_others), so the
scalar engine never reloads tables mid-kernel (the old Reciprocal function
lives in its own set and caused 32 x 1.3us ACT_TABLE_LOAD stalls on the
exp critical path). QKV/FC biases are added on the DVE during the
PSUM->SBUF copy (tensor_scalar with a per-partition [128,1] bias AP, or a
precomputed broadcast tile for V whose bias varies along the free dim) --
the old ones-vector bias matmuls burned ~27us of PE streaming+LDWEIGHTS.
The broadcast of 1/denom across partitions uses a selector matmul
(partition-shifted DVE/DMA ops misbehave; SBUF-side DMA access patterns
must keep the partition dim outermost). Mask-multiplies and normalize
multiplies run on the otherwise-idle GPSIMD engine (it cannot read PSUM);
diagonal-tile exp+mask handle both heads in one instruction via strided 3D
APs (mask tile duplicated per head). Normalization runs per query-chunk
(one chunk behind attention) and stages its praw slice into the AllToAll
DRAM buffer immediately, so the collective trigger's DMA wait is nearly
zero and the next batch's gpsimd mask-muls are not stalled behind it.
One AllToAll per batch (smaller per-half collectives measured ~3x slower
per byte), QKV of batch b+1 / FC of batch b-1 interleave into batch b's
attention to keep the PE fed while the scalar engine runs exp, and FC of
batch 2 is deferred into the tail to overlap the final AllToAll.
"""
import sys

for _p in ("/opt/trn_rl_repo",):
    if _p not in sys.path:
        sys.path.insert(0, _p)

import numpy as np

import concourse.bass as bass
import concourse.mybir as mybir
import concourse.tile as tile
from concourse import bacc
from concourse.bass_utils import run_bass_kernel_spmd

f32 = mybir.dt.float32
bf16 = mybir.dt.bfloat16
EXP = mybir.ActivationFunctionType.Exp
LN = mybir.ActivationFunctionType.Ln

B, T, D, H, HD = 4, 2048, 1024, 16, 64
NCORES = 8
HPC = H // NCORES          # heads per core = 2
BT = B * T                 # 8192
CH = 512                   # token chunk (q chunk / projection chunk)
NCH_B = T // CH            # 4 projection chunks per batch
QC = T // CH               # 4 query chunks per batch
NKV_B = T // 128           # 16 kv tiles of 128 per batch
TOK = 256                  # tokens per core per batch (after AllToAll)
SCALE = 1.0 / 8.0          # 1/sqrt(HD)

_CACHE = {}


class _Bacc(bacc.Bacc):
    """Bacc whose activation-table pass resolves Exp AND Ln to the combined
    natural_log_exp_and_others set (canonical id 6) instead of their
    individual default sets, so alternating Exp/Ln activations trigger one
    ACT_TABLE_LOAD total instead of one per transition (1.3us each, on the
    softmax critical path)."""

    def insert_act_table_loads(self):
        has_activation = any(
            isinstance(i, mybir.InstActivation)
            for b in self.main_func.blocks
            for i in b.instructions
        )
        if not has_activation:
            return
        from concourse.hw_specs import get_activation_tables
        tables = []
        for name, funcs in get_activation_tables(self.m.arch).items():
            funcs = set(funcs)
            if name == "exp_and_others":
                funcs.discard(mybir.ActivationFunctionType.Exp)
            if name == "natural_log":
                funcs.discard(mybir.ActivationFunctionType.Ln)
            tables.append((name, funcs))
        bacc._bass_rust.insert_act_table_loads(self, tables)


def _build(no_collective=False):
    nc = _Bacc("TRN2", target_bir_lowering=False, debug=False,
               num_devices=NCORES)

    xT = nc.dram_tensor("xT", [D, BT], bf16, kind="ExternalInput").ap()
    wq_d = nc.dram_tensor("wq", [128, 8 * 384], bf16, kind="ExternalInput").ap()
    bqc_d = nc.dram_tensor("bqc", [128, 2], f32, kind="ExternalInput").ap()
    vbb_d = nc.dram_tensor("vbb", [128, 128], bf16, kind="ExternalInput").ap()
    wfc_d = nc.dram_tensor("wfc", [128, 64 * 128], bf16,
                           kind="ExternalInput").ap()
    bfcc_d = nc.dram_tensor("bfcc", [128, 8], f32, kind="ExternalInput").ap()
    hm_d = nc.dram_tensor("hm", [128, 2 * 896], bf16, kind="ExternalInput").ap()
    outT = nc.dram_tensor("outT", [D, B * TOK], f32,
                          kind="ExternalOutput").ap()

    with tile.TileContext(nc) as tc:
        with tc.tile_pool(name="const", bufs=1) as cst, \
             tc.tile_pool(name="dram", bufs=1, space="DRAM") as dpool, \
             tc.tile_pool(name="work", bufs=1) as wk, \
             tc.tile_pool(name="ps", bufs=1, space="PSUM") as ps:

            # ---- weights needed first (QKV of batch 0) ----
            wq = cst.tile([128, 8 * 384], bf16)
            nc.sync.dma_start(wq[:], wq_d[:])
            bqc = cst.tile([128, 2], f32)
            nc.sync.dma_start(bqc[:], bqc_d[:])
            vbb = cst.tile([128, 128], bf16)
            nc.sync.dma_start(vbb[:], vbb_d[:])
            onesb = cst.tile([1, CH], bf16)
            nc.gpsimd.memset(onesb[:], 1.0)
            # selector for reciprocal broadcast: row 64 = 1, rest 0
            zl = cst.tile([65, 64], bf16)
            nc.gpsimd.memset(zl[:], 0.0)
            nc.gpsimd.memset(zl[64:65, :], 1.0)

            # ---- deferred constants (needed later; don't block first mm) ----
            def _late_consts():
                hm = cst.tile([128, 2 * 896], bf16)
                nc.sync.dma_start(hm[:], hm_d[:])
                wfc = cst.tile([128, 64 * 128], bf16)
                nc.sync.dma_start(wfc[:], wfc_d[:])
                bfcc = cst.tile([128, 8], f32)
                nc.sync.dma_start(bfcc[:], bfcc_d[:])
                # per-head reciprocal staging: row 64 = recip, rows 0-63
                # zeroed once (garbage would poison the selector matmul)
                rc = []
                for h in range(HPC):
                    t = cst.tile([65, T], bf16, name=f"rc{h}")
                    nc.gpsimd.memset(t[0:64, :], 0.0)
                    rc.append(t)
                return hm, wfc, bfcc, rc

            # ---- per-batch state (double buffered across the pipeline) ----
            def alloc_batch(b):
                qt = wk.tile([128, T], bf16, tag="qt", bufs=2, name=f"qt{b}")
                kt = wk.tile([128, T], bf16, tag="kt", bufs=2, name=f"kt{b}")
                vsb = wk.tile([128, NKV_B * 130], bf16, tag="vsb", bufs=2,
                              name=f"vsb{b}")
                v3 = vsb.rearrange("p (t c) -> p t c", c=130)
                nc.gpsimd.memset(v3[:, :, 64:65], 1.0)
                nc.gpsimd.memset(v3[:, :, 129:130], 1.0)
                praw = [wk.tile([64, T], bf16, tag=f"praw{h}", bufs=2,
                                name=f"praw{h}_{b}") for h in range(HPC)]
                return qt, kt, vsb, praw

            def qkv_dma(b, ch):
                """Prefetch one 512-token x chunk."""
                c0 = b * T + ch * CH
                xt = wk.tile([128, 8 * CH], bf16, tag="xt", bufs=4,
                             name=f"xt{b}_{ch}")
                xt3 = xt.rearrange("p (d c) -> p d c", d=8)
                xs3 = xT[:, c0:c0 + CH].rearrange("(d p) c -> p d c", p=128)
                nc.sync.dma_start(xt3[:], xs3)
                return xt

            def qkv_units(b, ch, xt, st):
                """Chunk projection as self-contained PE units (aux PSUM).

                Each unit allocates its own short-lived [128,512] aux tile so
                units can interleave with attention without pinning the
                st-tag rotation."""
                qt, kt, vsb = st[0], st[1], st[2]
                cs = ch * CH
                xt3 = xt.rearrange("p (d c) -> p d c", d=8)

                def q_unit(tgt, wo, bcol):
                    def emit():
                        pq = ps.tile([128, CH], f32, tag="aux", bufs=2,
                                     name=f"pq{b}_{ch}_{wo}")
                        for d in range(8):
                            nc.tensor.matmul(pq[:],
                                             wq[:, d * 384 + wo:d * 384 + wo + 128],
                                             xt[:, d * CH:(d + 1) * CH],
                                             start=(d == 0), stop=(d == 7))
                        nc.vector.tensor_scalar_add(
                            tgt[:, cs:cs + CH], pq[:], bqc[:, bcol:bcol + 1])
                    return emit

                def v_unit(sb):
                    def emit():
                        kvt = ch * 4 + sb
                        pvv = ps.tile([128, CH], f32, tag="aux", bufs=2,
                                      name=f"pvv{b}_{ch}_{sb}")
                        for d in range(8):
                            nc.tensor.matmul(
                                pvv[:, 0:128],
                                xt3[:, d, sb * 128:(sb + 1) * 128],
                                wq[:, d * 384 + 256:d * 384 + 384],
                                start=(d == 0), stop=(d == 7))
                        base = kvt * 130
                        nc.vector.tensor_add(vsb[:, base:base + 64],
                                             pvv[:, 0:64], vbb[:, 0:64])
                        nc.vector.tensor_add(vsb[:, base + 65:base + 129],
                                             pvv[:, 64:128], vbb[:, 64:128])
                    return emit

                return [q_unit(qt, 0, 0), q_unit(kt, 128, 1),
                        v_unit(0), v_unit(1), v_unit(2), v_unit(3)]

            def attn_qc(b, qc, st, hm, rc, drain):
                """Causal attention for query chunk qc of batch b."""
                qt, kt, vsb, praw = st
                nkv = 4 * (qc + 1)
                pv = [ps.tile([65, CH], f32, tag=f"pv{h}", bufs=1,
                              name=f"pv{h}_{b}_{qc}") for h in range(HPC)]
                hm3 = hm.rearrange("p (h c) -> p h c", h=2)
                for ki in range(nkv):
                    diag = ki - 4 * qc
                    # on diagonal tiles, queries below the block are fully
                    # masked: shrink the moving dim to the causal range
                    off = 128 * diag if diag > 0 else 0
                    m = CH - off
                    stt = ps.tile([128, 2 * CH], f32, tag="st", bufs=2,
                                  name=f"s_{b}_{qc}_{ki}")
                    pt = wk.tile([128, 2 * CH], bf16, tag="pt", bufs=3,
                                 name=f"p_{b}_{qc}_{ki}")
                    for h in range(HPC):
                        nc.tensor.matmul(
                            stt[:, h * CH + off:(h + 1) * CH],
                            kt[64 * h:64 * h + 64, ki * 128:(ki + 1) * 128],
                            qt[64 * h:64 * h + 64,
                               qc * CH + off:(qc + 1) * CH],
                            start=True, stop=True,
                            tile_position=(64 * h, 0))
                    # fill the scores->exp->PV latency with independent PE
                    # work: the unit lands between S(ki) and PV(ki) in the
                    # in-order PE queue, so PV no longer stalls on exp
                    drain(1)
                    if diag > 0:
                        st3 = stt.rearrange("p (h c) -> p h c", h=2)
                        pt3 = pt.rearrange("p (h c) -> p h c", h=2)
                        nc.scalar.activation(pt3[:, :, off:CH],
                                             st3[:, :, off:CH],
                                             EXP, scale=SCALE)
                    else:
                        nc.scalar.activation(pt[:], stt[:], EXP, scale=SCALE)
                    if diag >= 0:
                        pt3 = pt.rearrange("p (h c) -> p h c", h=2)
                        nc.gpsimd.tensor_mul(pt3[:, :, off:CH],
                                             pt3[:, :, off:CH],
                                             hm3[:, :, 384:384 + m])
                    for h in range(HPC):
                        vb = ki * 130 + 65 * h
                        nc.tensor.matmul(pv[h][0:65, off:CH],
                                         vsb[:, vb:vb + 65],
                                         pt[:, h * CH + off:(h + 1) * CH],
                                         start=(ki == 0),
                                         stop=(ki == nkv - 1))
                # stash raw PV + 1/denominator = exp(-ln(denom)); Ln+Exp
                # share one table set so no ACT_TABLE_LOAD is triggered.
                # ln intermediate kept f32 (bf16 would cost ~2% in exp).
                for h in range(HPC):
                    nc.vector.tensor_copy(praw[h][:, qc * CH:(qc + 1) * CH],
                                          pv[h][0:64, :])
                    lnt = wk.tile([1, CH], f32, tag="lnt", bufs=2,
                                  name=f"lnt{h}_{b}_{qc}")
                    nc.scalar.activation(lnt[:], pv[h][64:65, :], LN)
                    nc.scalar.activation(rc[h][64:65, qc * CH:(qc + 1) * CH],
                                         lnt[:], EXP, scale=-1.0)

            def normalize_qc(b, qc, st, rc, ag_in):
                """praw[:, qc] *= broadcast(1/denom), then stage into the
                AllToAll input so the collective trigger has ~no DMA wait."""
                praw = st[3]
                div = ag_in.rearrange("(d p) c -> p d c", p=128)
                for h in range(HPC):
                    bcq = ps.tile([128, CH], f32, tag="aux", bufs=2,
                                  name=f"bc{h}_{b}_{qc}")
                    nc.tensor.matmul(bcq[0:64, :], zl[:, 0:64],
                                     rc[h][:, qc * CH:(qc + 1) * CH],
                                     start=True, stop=True)
                    rbs = wk.tile([64, CH], bf16, tag="rbs", bufs=2,
                                  name=f"rbs{h}_{b}_{qc}")
                    nc.vector.tensor_copy(rbs[:], bcq[0:64, :])
                    nc.gpsimd.tensor_mul(
                        praw[h][:, qc * CH:(qc + 1) * CH],
                        praw[h][:, qc * CH:(qc + 1) * CH], rbs[:])
                    src = praw[h][:, qc * CH:(qc + 1) * CH].rearrange(
                        "p (d c) -> p d c", c=TOK)
                    nc.sync.dma_start(
                        div[64 * h:64 * h + 64, 2 * qc:2 * qc + 2, :], src)

            def a2a_alloc(b):
                ag_in = dpool.tile([NCORES * 128, TOK], bf16,
                                   name=f"ag_in{b}")
                ag_out = dpool.tile([NCORES * 128, TOK], bf16,
                                    name=f"ag_out{b}")
                return ag_in, ag_out

            def a2a_batch(b, ag_in, ag_out):
                """Re-shard batch b attention output: head- to token-sharded."""
                if no_collective:
                    nc.sync.dma_start(ag_out[:], ag_in[:])
                else:
                    nc.gpsimd.collective_compute(
                        "AllToAll", mybir.AluOpType.bypass,
                        replica_groups=[list(range(NCORES))],
                        ins=[ag_in.opt()], outs=[ag_out.opt()])
                return ag_out

            def fc_units(b, ag_out, wfc, bfcc):
                """Full FC for this core's 256-token slice, as PE units."""
                box = {}

                def load():
                    fci = wk.tile([128, 8 * TOK], bf16, tag="fci", bufs=2,
                                  name=f"fci{b}")
                    fci3 = fci.rearrange("p (d c) -> p d c", d=8)
                    srcv = ag_out.rearrange("(d p) c -> p d c", p=128)
                    nc.sync.dma_start(fci3[:], srcv)
                    box["fci"] = fci
                    box["ost"] = wk.tile([128, 8 * TOK], f32, tag="ost",
                                         bufs=2, name=f"ost{b}")

                def fo_unit(fo):
                    def emit():
                        fci, ost = box["fci"], box["ost"]
                        pfc = ps.tile([128, CH], f32, tag="aux", bufs=2,
                                      name=f"pfc{b}_{fo}")
                        for d in range(8):
                            nc.tensor.matmul(
                                pfc[:, 0:TOK],
                                wfc[:, (fo * 8 + d) * 128:
                                     (fo * 8 + d + 1) * 128],
                                fci[:, d * TOK:(d + 1) * TOK],
                                start=(d == 0), stop=(d == 7))
                        nc.vector.tensor_scalar_add(
                            ost[:, fo * TOK:(fo + 1) * TOK], pfc[:, 0:TOK],
                            bfcc[:, fo:fo + 1])
                    return emit

                def store():
                    dst = outT.rearrange("(f p) c -> p f c", p=128)[
                        :, :, b * TOK:(b + 1) * TOK]
                    osrc = box["ost"].rearrange("p (f c) -> p f c", c=TOK)
                    nc.sync.dma_start(dst, osrc)

                return [load] + [fo_unit(fo) for fo in range(8)] + [store]

            def dummy_unit(tag_i):
                """~0.9us of dependency-free matmuls to keep the HAM warm."""
                def emit():
                    pdm = ps.tile([128, CH], f32, tag="aux", bufs=2,
                                  name=f"pdm{tag_i}")
                    for r in range(4):
                        nc.tensor.matmul(pdm[0:1, :], onesb[0:1, 0:1],
                                         onesb[0:1, :],
                                         start=True, stop=True)
                return emit

            # ================= schedule =================
            from collections import deque
            filler = deque()

            def drain(n):
                for _ in range(min(n, len(filler))):
                    filler.popleft()()

            def drain_all():
                while filler:
                    filler.popleft()()

            states = [None] * B
            states[0] = alloc_batch(0)
            xts = {(0, 0): qkv_dma(0, 0)}
            hm, wfc, bfcc, rc = _late_consts()
            # warm up the collective stream so the first real AllToAll does
            # not absorb the cross-core barrier + algorithm setup (~20us)
            if not no_collective:
                w_in = dpool.tile([NCORES, 16], bf16, name="warm_in")
                w_out = dpool.tile([NCORES, 16], bf16, name="warm_out")
                wsb = cst.tile([8, 16], bf16, name="warm_sb")
                nc.gpsimd.memset(wsb[:], 0.0)
                nc.sync.dma_start(w_in[:], wsb[:])
                nc.gpsimd.collective_compute(
                    "AllToAll", mybir.AluOpType.bypass,
                    replica_groups=[list(range(NCORES))],
                    ins=[w_in.opt()], outs=[w_out.opt()])
            # preloop: only chunks 0/1 of batch 0 run serially; its chunks
            # 2/3 become filler inside batch 0's own qc0/qc1
            xts[(0, 1)] = qkv_dma(0, 1)
            for ch in (0, 1):
                for u in qkv_units(0, ch, xts.pop((0, ch)), states[0]):
                    u()
            xts[(0, 2)] = qkv_dma(0, 2)
            xts[(0, 3)] = qkv_dma(0, 3)

            ags = [a2a_alloc(b) for b in range(B)]
            ndum = 0
            # QKV of batch X is staggered: chunks 0/1 run as filler during
            # batch X-1 (qc2/qc3), chunks 2/3 during batch X itself
            # (qc0/qc1).  This gives EVERY batch -- including the last --
            # ~10us of real PE filler in its first half, where previously
            # batch 3 had nothing and the PE micro-idled between S and PV
            # long enough for the HAM to hold the clock at K=4/8 for the
            # whole final quarter of the kernel.  Every chunk's x DMA is
            # issued >= 2 qc slots before its units drain, so the first
            # projection matmul of a batch never waits on HBM.
            for b in range(B):
                for qc in range(QC):
                    if qc in (0, 1):
                        filler.extend(qkv_units(b, qc + 2,
                                                xts.pop((b, qc + 2)),
                                                states[b]))
                        if b + 1 < B:
                            if qc == 0:
                                states[b + 1] = alloc_batch(b + 1)
                            xts[(b + 1, qc)] = qkv_dma(b + 1, qc)
                    else:
                        if b + 1 < B:
                            filler.extend(qkv_units(b + 1, qc - 2,
                                                    xts.pop((b + 1, qc - 2)),
                                                    states[b + 1]))
                            xts[(b + 1, qc)] = qkv_dma(b + 1, qc)
                    if qc == 2 and b in (1, 2):
                        filler.extend(fc_units(b - 1, ags[b - 1][1],
                                               wfc, bfcc))
                    if b == B - 1 and qc >= 2:
                        # no next-batch QKV to interleave: keep PE warm
                        for _ in range(6):
                            ndum += 1
                            filler.append(dummy_unit(ndum))
                    attn_qc(b, qc, states[b], hm, rc, drain)
                    if qc >= 1:
                        normalize_qc(b, qc - 1, states[b], rc, ags[b][0])
                drain_all()
                normalize_qc(b, QC - 1, states[b], rc, ags[b][0])
                a2a_batch(b, *ags[b])
            # tail: FC(2) was deferred here so real PE work (instead of
            # dummies) covers the AllToAll(3) completion window
            for _ in range(4):
                ndum += 1
                dummy_unit(ndum)()
            for u in fc_units(B - 2, ags[B - 2][1], wfc, bfcc):
                u()
            for _ in range(2):
                ndum += 1
                dummy_unit(ndum)()
            for u in fc_units(B - 1, ags[B - 1][1], wfc, bfcc):
                u()

    nc.compile()
    return nc


def _host_inputs(x, W_qkv, b_qkv, W_fc, b_fc):
    import ml_dtypes
    bf = ml_dtypes.bfloat16
    x = np.asarray(x, dtype=np.float32)
    W_qkv = np.asarray(W_qkv, dtype=np.float32)
    b_qkv = np.asarray(b_qkv, dtype=np.float32)
    W_fc = np.asarray(W_fc, dtype=np.float32)
    b_fc = np.asarray(b_fc, dtype=np.float32)

    xT = np.ascontiguousarray(x.reshape(BT, D).T).astype(bf)
    hm1 = (np.arange(128)[:, None]
           <= np.arange(896)[None, :] - 384).astype(bf)
    hm = np.ascontiguousarray(np.concatenate([hm1, hm1], axis=1))
    # full FC weights prepacked to SBUF layout [p, (f*8+d)*128 + c]
    wfc = np.ascontiguousarray(
        W_fc.reshape(8, 128, 8, 128).transpose(1, 2, 0, 3).reshape(128, 8192)
    ).astype(bf)
    # FC bias: column fo = features fo*128..(fo+1)*128 (per-partition)
    bfcc = np.ascontiguousarray(b_fc.reshape(8, 128).T).astype(np.float32)
    in_maps = []
    for c in range(NCORES):
        f0 = c * (HPC * HD)  # 128*c
        wqs = np.concatenate(
            [W_qkv[:, p * D + f0: p * D + f0 + 128] for p in range(3)],
            axis=1)  # [1024, 384] = [q|k|v]
        wq_c = np.ascontiguousarray(
            wqs.reshape(8, 128, 384).transpose(1, 0, 2).reshape(128, 3072)
        ).astype(bf)
        # q/k bias as per-partition columns [128, 2]
        bqc_c = np.ascontiguousarray(np.stack(
            [b_qkv[f0:f0 + 128], b_qkv[D + f0:D + f0 + 128]], axis=1)
        ).astype(np.float32)
        # v bias broadcast tile [128 rows (tokens), 128 cols (features)]
        vbb_c = np.ascontiguousarray(np.broadcast_to(
            b_qkv[2 * D + f0:2 * D + f0 + 128][None, :], (128, 128))
        ).astype(bf)
        in_maps.append({
            "xT": xT, "wq": wq_c, "bqc": bqc_c, "vbb": vbb_c, "wfc": wfc,
            "bfcc": bfcc, "hm": hm,
        })
    return in_maps


def _get_nc():
    if "nc" not in _CACHE:
        _CACHE["nc"] = _build()
    return _CACHE["nc"]


def _assemble(results):
    full = np.empty((BT, D), dtype=np.float32)
    for c in range(NCORES):
        o = results[c]["outT"]  # [1024 features, 4*256 tokens]
        for b in range(B):
            full[b * T + c * TOK: b * T + (c + 1) * TOK, :] = \
                o[:, b * TOK:(b + 1) * TOK].T
    return full.reshape(B, T, D)


def kernel(x, W_qkv, b_qkv, W_fc, b_fc):
    nc = _get_nc()
    in_maps = _host_inputs(x, W_qkv, b_qkv, W_fc, b_fc)
    res = run_bass_kernel_spmd(nc, in_maps, list(range(NCORES)))
    return _assemble(res.results)


# revision 9
# speedup vs baseline: 1.2056x; 1.0785x over previous
"""Trainium2 Bass kernel for causal multi-head attention (B=4, T=2048, D=1024, H=16).

Sharding: tensor-parallel over heads for QKV+attention (each of 8 cores owns
2 heads over all tokens), then AllToAll re-shards from head-sharded to
token-sharded so each core computes the final FC over the full feature dim
for its 256-token slice of each batch.

All matmuls run in bf16 (fp32 streams at half PE rate; bf16 at full), with
fp32 PSUM accumulation. Scores are computed transposed (S^T = K Q^T, two
heads packed in PE quadrants via tile_position) so softmax normalization
lands on the PV matmul's free dim; the denominator comes from a ones column
augmented into V. Normalization is deferred out of the inner loop (the
per-chunk broadcast chain caused PE bubbles + HAM re-throttling). The
denominator reciprocal is computed as exp(-ln(x)) on the scalar engine: Ln
and Exp share one activation table set (natural_log_exp_and_others), so the
scalar engine never reloads tables mid-kernel (the old Reciprocal function
lives in its own set and caused 32 x 1.3us ACT_TABLE_LOAD stalls on the
exp critical path). QKV/FC biases are added on the DVE during the
PSUM->SBUF copy (tensor_scalar with a per-partition [128,1] bias AP, or a
precomputed broadcast tile for V whose bias varies along the free dim) --
the old ones-vector bias matmuls burned ~27us of PE streaming+LDWEIGHTS.
The broadcast of 1/denom across partitions uses a selector matmul
(partition-shifted DVE/DMA ops misbehave; SBUF-side DMA access patterns
must keep the partition dim outermost). Mask-multiplies and normalize
multiplies run on the otherwise-idle GPSIMD engine (it cannot read PSUM);
diagonal-tile exp+mask handle both heads in one instruction via strided 3D
APs (mask tile duplicated per head). Normalization runs per query-chunk
(one chunk behind attention) and stages its praw slice into the AllToAll
DRAM buffer immediately, so the collective trigger's DMA wait is nearly
zero and the next batch's gpsimd mask-muls are not stalled behind it.
One AllToAll per batch (smaller per-half collectives measured ~3x slower
per byte), QKV of batch b+1 / FC of batch b-1 interleave into batch b's
attention to keep the PE fed while the scalar engine runs exp, and FC of
batch 2 is deferred into the tail to overlap the final AllToAll.
"""
import sys

for _p in ("/opt/trn_rl_repo",):
    if _p not in sys.path:
        sys.path.insert(0, _p)

import numpy as np

import concourse.bass as bass
import concourse.mybir as mybir
import concourse.tile as tile
from concourse import bacc
from concourse.bass_utils import run_bass_kernel_spmd

f32 = mybir.dt.float32
bf16 = mybir.dt.bfloat16
EXP = mybir.ActivationFunctionType.Exp
LN = mybir.ActivationFunctionType.Ln

B, T, D, H, HD = 4, 2048, 1024, 16, 64
NCORES = 8
HPC = H // NCORES          # heads per core = 2
BT = B * T                 # 8192
CH = 512                   # token chunk (q chunk / projection chunk)
NCH_B = T // CH            # 4 projection chunks per batch
QC = T // CH               # 4 query chunks per batch
NKV_B = T // 128           # 16 kv tiles of 128 per batch
TOK = 256                  # tokens per core per batch (after AllToAll)
SCALE = 1.0 / 8.0          # 1/sqrt(HD)

_CACHE = {}


class _Bacc(bacc.Bacc):
    """Bacc whose activation-table pass resolves Exp AND Ln to the combined
    natural_log_exp_and_others set (canonical id 6) instead of their
    individual default sets, so alternating Exp/Ln activations trigger one
    ACT_TABLE_LOAD total instead of one per transition (1.3us each, on the
    softmax critical path)."""

    def insert_act_table_loads(self):
        has_activation = any(
            isinstance(i, mybir.InstActivation)
            for b in self.main_func.blocks
            for i in b.instructions
        )
        if not has_activation:
            return
        from concourse.hw_specs import get_activation_tables
        tables = []
        for name, funcs in get_activation_tables(self.m.arch).items():
            funcs = set(funcs)
            if name == "exp_and_others":
                funcs.discard(mybir.ActivationFunctionType.Exp)
            if name == "natural_log":
                funcs.discard(mybir.ActivationFunctionType.Ln)
            tables.append((name, funcs))
        bacc._bass_rust.insert_act_table_loads(self, tables)


def _build(no_collective=False):
    nc = _Bacc("TRN2", target_bir_lowering=False, debug=False,
               num_devices=NCORES)

    xT = nc.dram_tensor("xT", [D, BT], bf16, kind="ExternalInput").ap()
    wq_d = nc.dram_tensor("wq", [128, 8 * 384], bf16, kind="ExternalInput").ap()
    bqc_d = nc.dram_tensor("bqc", [128, 2], f32, kind="ExternalInput").ap()
    vbb_d = nc.dram_tensor("vbb", [128, 128], bf16, kind="ExternalInput").ap()
    wfc_d = nc.dram_tensor("wfc", [128, 64 * 128], bf16,
                           kind="ExternalInput").ap()
    bfcc_d = nc.dram_tensor("bfcc", [128, 8], f32, kind="ExternalInput").ap()
    hm_d = nc.dram_tensor("hm", [128, 2 * 896], bf16, kind="ExternalInput").ap()
    outT = nc.dram_tensor("outT", [D, B * TOK], f32,
                          kind="ExternalOutput").ap()

    with tile.TileContext(nc) as tc:
        with tc.tile_pool(name="const", bufs=1) as cst, \
             tc.tile_pool(name="dram", bufs=1, space="DRAM") as dpool, \
             tc.tile_pool(name="work", bufs=1) as wk, \
             tc.tile_pool(name="ps", bufs=1, space="PSUM") as ps:

            # ---- weights needed first (QKV of batch 0) ----
            wq = cst.tile([128, 8 * 384], bf16)
            nc.sync.dma_start(wq[:], wq_d[:])
            bqc = cst.tile([128, 2], f32)
            nc.sync.dma_start(bqc[:], bqc_d[:])
            vbb = cst.tile([128, 128], bf16)
            nc.sync.dma_start(vbb[:], vbb_d[:])
            onesb = cst.tile([1, CH], bf16)
            nc.gpsimd.memset(onesb[:], 1.0)
            # selector for reciprocal broadcast: row 64 = 1, rest 0
            zl = cst.tile([65, 64], bf16)
            nc.gpsimd.memset(zl[:], 0.0)
            nc.gpsimd.memset(zl[64:65, :], 1.0)

            # ---- deferred constants (needed later; don't block first mm) ----
            def _late_consts():
                hm = cst.tile([128, 2 * 896], bf16)
                nc.sync.dma_start(hm[:], hm_d[:])
                wfc = cst.tile([128, 64 * 128], bf16)
                nc.sync.dma_start(wfc[:], wfc_d[:])
                bfcc = cst.tile([128, 8], f32)
                nc.sync.dma_start(bfcc[:], bfcc_d[:])
                # per-head reciprocal staging: row 64 = recip, rows 0-63
                # zeroed once (garbage would poison the selector matmul)
                rc = []
                for h in range(HPC):
                    t = cst.tile([65, T], bf16, name=f"rc{h}")
                    nc.gpsimd.memset(t[0:64, :], 0.0)
                    rc.append(t)
                return hm, wfc, bfcc, rc

            # ---- per-batch state (double buffered across the pipeline) ----
            def alloc_batch(b):
                qt = wk.tile([128, T], bf16, tag="qt", bufs=2, name=f"qt{b}")
                kt = wk.tile([128, T], bf16, tag="kt", bufs=2, name=f"kt{b}")
                vsb = wk.tile([128, NKV_B * 130], bf16, tag="vsb", bufs=2,
                              name=f"vsb{b}")
                v3 = vsb.rearrange("p (t c) -> p t c", c=130)
                nc.gpsimd.memset(v3[:, :, 64:65], 1.0)
                nc.gpsimd.memset(v3[:, :, 129:130], 1.0)
                praw = [wk.tile([64, T], bf16, tag=f"praw{h}", bufs=2,
                                name=f"praw{h}_{b}") for h in range(HPC)]
                return qt, kt, vsb, praw

            def qkv_dma(b, ch):
                """Prefetch one 512-token x chunk."""
                c0 = b * T + ch * CH
                xt = wk.tile([128, 8 * CH], bf16, tag="xt", bufs=4,
                             name=f"xt{b}_{ch}")
                xt3 = xt.rearrange("p (d c) -> p d c", d=8)
                xs3 = xT[:, c0:c0 + CH].rearrange("(d p) c -> p d c", p=128)
                nc.sync.dma_start(xt3[:], xs3)
                return xt

            def qkv_units(b, ch, xt, st):
                """Chunk projection as self-contained PE units (aux PSUM).

                Each unit allocates its own short-lived [128,512] aux tile so
                units can interleave with attention without pinning the
                st-tag rotation."""
                qt, kt, vsb = st[0], st[1], st[2]
                cs = ch * CH
                xt3 = xt.rearrange("p (d c) -> p d c", d=8)

                def q_unit(tgt, wo, bcol):
                    def emit():
                        pq = ps.tile([128, CH], f32, tag="aux", bufs=2,
                                     name=f"pq{b}_{ch}_{wo}")
                        for d in range(8):
                            nc.tensor.matmul(pq[:],
                                             wq[:, d * 384 + wo:d * 384 + wo + 128],
                                             xt[:, d * CH:(d + 1) * CH],
                                             start=(d == 0), stop=(d == 7))
                        nc.vector.tensor_scalar_add(
                            tgt[:, cs:cs + CH], pq[:], bqc[:, bcol:bcol + 1])
                    return emit

                def v_unit(sb):
                    def emit():
                        kvt = ch * 4 + sb
                        pvv = ps.tile([128, CH], f32, tag="aux", bufs=2,
                                      name=f"pvv{b}_{ch}_{sb}")
                        for d in range(8):
                            nc.tensor.matmul(
                                pvv[:, 0:128],
                                xt3[:, d, sb * 128:(sb + 1) * 128],
                                wq[:, d * 384 + 256:d * 384 + 384],
                                start=(d == 0), stop=(d == 7))
                        base = kvt * 130
                        nc.vector.tensor_add(vsb[:, base:base + 64],
                                             pvv[:, 0:64], vbb[:, 0:64])
                        nc.vector.tensor_add(vsb[:, base + 65:base + 129],
                                             pvv[:, 64:128], vbb[:, 64:128])
                    return emit

                return [q_unit(qt, 0, 0), q_unit(kt, 128, 1),
                        v_unit(0), v_unit(1), v_unit(2), v_unit(3)]

            def attn_qc(b, qc, st, hm, rc, drain):
                """Causal attention for query chunk qc of batch b."""
                qt, kt, vsb, praw = st
                nkv = 4 * (qc + 1)
                pv = [ps.tile([65, CH], f32, tag=f"pv{h}", bufs=1,
                              name=f"pv{h}_{b}_{qc}") for h in range(HPC)]
                hm3 = hm.rearrange("p (h c) -> p h c", h=2)
                for ki in range(nkv):
                    diag = ki - 4 * qc
                    # on diagonal tiles, queries below the block are fully
                    # masked: shrink the moving dim to the causal range
                    off = 128 * diag if diag > 0 else 0
                    m = CH - off
                    stt = ps.tile([128, 2 * CH], f32, tag="st", bufs=2,
                                  name=f"s_{b}_{qc}_{ki}")
                    pt = wk.tile([128, 2 * CH], bf16, tag="pt", bufs=3,
                                 name=f"p_{b}_{qc}_{ki}")
                    for h in range(HPC):
                        nc.tensor.matmul(
                            stt[:, h * CH + off:(h + 1) * CH],
                            kt[64 * h:64 * h + 64, ki * 128:(ki + 1) * 128],
                            qt[64 * h:64 * h + 64,
                               qc * CH + off:(qc + 1) * CH],
                            start=True, stop=True,
                            tile_position=(64 * h, 0))
                    # fill the scores->exp->PV latency with independent PE
                    # work: the unit lands between S(ki) and PV(ki) in the
                    # in-order PE queue, so PV no longer stalls on exp
                    drain(1)
                    if diag > 0:
                        st3 = stt.rearrange("p (h c) -> p h c", h=2)
                        pt3 = pt.rearrange("p (h c) -> p h c", h=2)
                        nc.scalar.activation(pt3[:, :, off:CH],
                                             st3[:, :, off:CH],
                                             EXP, scale=SCALE)
                    else:
                        nc.scalar.activation(pt[:], stt[:], EXP, scale=SCALE)
                    if diag >= 0:
                        # only columns [off, off+128) of a diagonal tile are
                        # actually triangular-masked (the rest are fully
                        # valid), so multiply just that 128-wide strip --
                        # 4x less gpsimd work on the exp->PV critical path
                        pt3 = pt.rearrange("p (h c) -> p h c", h=2)
                        nc.gpsimd.tensor_mul(pt3[:, :, off:off + 128],
                                             pt3[:, :, off:off + 128],
                                             hm3[:, :, 384:512])
                    for h in range(HPC):
                        vb = ki * 130 + 65 * h
                        if diag >= 0 and m > 128:
                            # split PV: the clean columns don't need to wait
                            # for the mask multiply on the masked strip
                            nc.tensor.matmul(
                                pv[h][0:65, off:off + 128],
                                vsb[:, vb:vb + 65],
                                pt[:, h * CH + off:h * CH + off + 128],
                                start=(ki == 0), stop=(ki == nkv - 1))
                            nc.tensor.matmul(
                                pv[h][0:65, off + 128:CH],
                                vsb[:, vb:vb + 65],
                                pt[:, h * CH + off + 128:(h + 1) * CH],
                                start=(ki == 0), stop=(ki == nkv - 1))
                        else:
                            nc.tensor.matmul(
                                pv[h][0:65, off:CH],
                                vsb[:, vb:vb + 65],
                                pt[:, h * CH + off:(h + 1) * CH],
                                start=(ki == 0),
                                stop=(ki == nkv - 1))
                # stash raw PV + 1/denominator = exp(-ln(denom)); Ln+Exp
                # share one table set so no ACT_TABLE_LOAD is triggered.
                # ln intermediate kept f32 (bf16 would cost ~2% in exp).
                for h in range(HPC):
                    nc.vector.tensor_copy(praw[h][:, qc * CH:(qc + 1) * CH],
                                          pv[h][0:64, :])
                    lnt = wk.tile([1, CH], f32, tag="lnt", bufs=2,
                                  name=f"lnt{h}_{b}_{qc}")
                    nc.scalar.activation(lnt[:], pv[h][64:65, :], LN)
                    nc.scalar.activation(rc[h][64:65, qc * CH:(qc + 1) * CH],
                                         lnt[:], EXP, scale=-1.0)

            def normalize_qc(b, qc, st, rc, ag_in):
                """praw[:, qc] *= broadcast(1/denom), then stage into the
                AllToAll input so the collective trigger has ~no DMA wait."""
                praw = st[3]
                div = ag_in.rearrange("(d p) c -> p d c", p=128)
                for h in range(HPC):
                    bcq = ps.tile([128, CH], f32, tag="aux", bufs=2,
                                  name=f"bc{h}_{b}_{qc}")
                    nc.tensor.matmul(bcq[0:64, :], zl[:, 0:64],
                                     rc[h][:, qc * CH:(qc + 1) * CH],
                                     start=True, stop=True)
                    rbs = wk.tile([64, CH], bf16, tag="rbs", bufs=2,
                                  name=f"rbs{h}_{b}_{qc}")
                    nc.vector.tensor_copy(rbs[:], bcq[0:64, :])
                    nc.gpsimd.tensor_mul(
                        praw[h][:, qc * CH:(qc + 1) * CH],
                        praw[h][:, qc * CH:(qc + 1) * CH], rbs[:])
                    src = praw[h][:, qc * CH:(qc + 1) * CH].rearrange(
                        "p (d c) -> p d c", c=TOK)
                    nc.sync.dma_start(
                        div[64 * h:64 * h + 64, 2 * qc:2 * qc + 2, :], src)

            def a2a_alloc(b):
                ag_in = dpool.tile([NCORES * 128, TOK], bf16,
                                   name=f"ag_in{b}")
                ag_out = dpool.tile([NCORES * 128, TOK], bf16,
                                    name=f"ag_out{b}")
                return ag_in, ag_out

            def a2a_batch(b, ag_in, ag_out):
                """Re-shard batch b attention output: head- to token-sharded."""
                if no_collective:
                    nc.sync.dma_start(ag_out[:], ag_in[:])
                else:
                    nc.gpsimd.collective_compute(
                        "AllToAll", mybir.AluOpType.bypass,
                        replica_groups=[list(range(NCORES))],
                        ins=[ag_in.opt()], outs=[ag_out.opt()])
                return ag_out

            def fc_units(b, ag_out, wfc, bfcc):
                """Full FC for this core's 256-token slice, as PE units."""
                box = {}

                def load():
                    fci = wk.tile([128, 8 * TOK], bf16, tag="fci", bufs=2,
                                  name=f"fci{b}")
                    fci3 = fci.rearrange("p (d c) -> p d c", d=8)
                    srcv = ag_out.rearrange("(d p) c -> p d c", p=128)
                    nc.sync.dma_start(fci3[:], srcv)
                    box["fci"] = fci
                    box["ost"] = wk.tile([128, 8 * TOK], f32, tag="ost",
                                         bufs=2, name=f"ost{b}")

                def fo_unit(fo):
                    def emit():
                        fci, ost = box["fci"], box["ost"]
                        pfc = ps.tile([128, CH], f32, tag="aux", bufs=2,
                                      name=f"pfc{b}_{fo}")
                        for d in range(8):
                            nc.tensor.matmul(
                                pfc[:, 0:TOK],
                                wfc[:, (fo * 8 + d) * 128:
                                     (fo * 8 + d + 1) * 128],
                                fci[:, d * TOK:(d + 1) * TOK],
                                start=(d == 0), stop=(d == 7))
                        nc.vector.tensor_scalar_add(
                            ost[:, fo * TOK:(fo + 1) * TOK], pfc[:, 0:TOK],
                            bfcc[:, fo:fo + 1])
                    return emit

                def store():
                    dst = outT.rearrange("(f p) c -> p f c", p=128)[
                        :, :, b * TOK:(b + 1) * TOK]
                    osrc = box["ost"].rearrange("p (f c) -> p f c", c=TOK)
                    nc.sync.dma_start(dst, osrc)

                return [load] + [fo_unit(fo) for fo in range(8)] + [store]

            def dummy_unit(tag_i):
                """~0.9us of dependency-free matmuls to keep the HAM warm."""
                def emit():
                    pdm = ps.tile([128, CH], f32, tag="aux", bufs=2,
                                  name=f"pdm{tag_i}")
                    for r in range(4):
                        nc.tensor.matmul(pdm[0:1, :], onesb[0:1, 0:1],
                                         onesb[0:1, :],
                                         start=True, stop=True)
                return emit

            # ================= schedule =================
            from collections import deque
            filler = deque()

            def drain(n):
                for _ in range(min(n, len(filler))):
                    filler.popleft()()

            def drain_all():
                while filler:
                    filler.popleft()()

            states = [None] * B
            states[0] = alloc_batch(0)
            xts = {(0, 0): qkv_dma(0, 0)}
            hm, wfc, bfcc, rc = _late_consts()
            # warm up the collective stream so the first real AllToAll does
            # not absorb the cross-core barrier + algorithm setup (~20us)
            if not no_collective:
                w_in = dpool.tile([NCORES, 16], bf16, name="warm_in")
                w_out = dpool.tile([NCORES, 16], bf16, name="warm_out")
                wsb = cst.tile([8, 16], bf16, name="warm_sb")
                nc.gpsimd.memset(wsb[:], 0.0)
                nc.sync.dma_start(w_in[:], wsb[:])
                nc.gpsimd.collective_compute(
                    "AllToAll", mybir.AluOpType.bypass,
                    replica_groups=[list(range(NCORES))],
                    ins=[w_in.opt()], outs=[w_out.opt()])
            # preloop: only chunks 0/1 of batch 0 run serially; its chunks
            # 2/3 become filler inside batch 0's own qc0/qc1
            xts[(0, 1)] = qkv_dma(0, 1)
            for ch in (0, 1):
                for u in qkv_units(0, ch, xts.pop((0, ch)), states[0]):
                    u()
            xts[(0, 2)] = qkv_dma(0, 2)
            xts[(0, 3)] = qkv_dma(0, 3)

            ags = [a2a_alloc(b) for b in range(B)]
            ndum = 0
            # QKV of batch X is staggered: chunks 0/1 run as filler during
            # batch X-1 (qc2/qc3), chunks 2/3 during batch X itself
            # (qc0/qc1).  This gives EVERY batch -- including the last --
            # ~10us of real PE filler in its first half, where previously
            # batch 3 had nothing and the PE micro-idled between S and PV
            # long enough for the HAM to hold the clock at K=4/8 for the
            # whole final quarter of the kernel.  Every chunk's x DMA is
            # issued >= 2 qc slots before its units drain, so the first
            # projection matmul of a batch never waits on HBM.
            for b in range(B):
                for qc in range(QC):
                    if qc in (0, 1):
                        filler.extend(qkv_units(b, qc + 2,
                                                xts.pop((b, qc + 2)),
                                                states[b]))
                        if b + 1 < B:
                            if qc == 0:
                                states[b + 1] = alloc_batch(b + 1)
                            xts[(b + 1, qc)] = qkv_dma(b + 1, qc)
                    else:
                        if b + 1 < B:
                            filler.extend(qkv_units(b + 1, qc - 2,
                                                    xts.pop((b + 1, qc - 2)),
                                                    states[b + 1]))
                            xts[(b + 1, qc)] = qkv_dma(b + 1, qc)
                    if qc == 2 and b in (1, 2):
                        filler.extend(fc_units(b - 1, ags[b - 1][1],
                                               wfc, bfcc))
                    if b == B - 1 and qc >= 2:
                        # no next-batch QKV to interleave: keep PE warm
                        for _ in range(6):
                            ndum += 1
                            filler.append(dummy_unit(ndum))
                    attn_qc(b, qc, states[b], hm, rc, drain)
                    if qc >= 1:
                        normalize_qc(b, qc - 1, states[b], rc, ags[b][0])
                drain_all()
                normalize_qc(b, QC - 1, states[b], rc, ags[b][0])
                a2a_batch(b, *ags[b])
            # tail: FC(2) was deferred here so real PE work (instead of
            # dummies) covers the AllToAll(3) completion window
            for _ in range(4):
                ndum += 1
                dummy_unit(ndum)()
            for u in fc_units(B - 2, ags[B - 2][1], wfc, bfcc):
                u()
            for _ in range(4):
                ndum += 1
                dummy_unit(ndum)()
            for u in fc_units(B - 1, ags[B - 1][1], wfc, bfcc):
                u()

    nc.compile()
    return nc


def _host_inputs(x, W_qkv, b_qkv, W_fc, b_fc):
    import ml_dtypes
    bf = ml_dtypes.bfloat16
    x = np.asarray(x, dtype=np.float32)
    W_qkv = np.asarray(W_qkv, dtype=np.float32)
    b_qkv = np.asarray(b_qkv, dtype=np.float32)
    W_fc = np.asarray(W_fc, dtype=np.float32)
    b_fc = np.asarray(b_fc, dtype=np.float32)

    xT = np.ascontiguousarray(x.reshape(BT, D).T).astype(bf)
    hm1 = (np.arange(128)[:, None]
           <= np.arange(896)[None, :] - 384).astype(bf)
    hm = np.ascontiguousarray(np.concatenate([hm1, hm1], axis=1))
    # full FC weights prepacked to SBUF layout [p, (f*8+d)*128 + c]
    wfc = np.ascontiguousarray(
        W_fc.reshape(8, 128, 8, 128).transpose(1, 2, 0, 3).reshape(128, 8192)
    ).astype(bf)
    # FC bias: column fo = features fo*128..(fo+1)*128 (per-partition)
    bfcc = np.ascontiguousarray(b_fc.reshape(8, 128).T).astype(np.float32)
    in_maps = []
    for c in range(NCORES):
        f0 = c * (HPC * HD)  # 128*c
        wqs = np.concatenate(
            [W_qkv[:, p * D + f0: p * D + f0 + 128] for p in range(3)],
            axis=1)  # [1024, 384] = [q|k|v]
        wq_c = np.ascontiguousarray(
            wqs.reshape(8, 128, 384).transpose(1, 0, 2).reshape(128, 3072)
        ).astype(bf)
        # q/k bias as per-partition columns [128, 2]
        bqc_c = np.ascontiguousarray(np.stack(
            [b_qkv[f0:f0 + 128], b_qkv[D + f0:D + f0 + 128]], axis=1)
        ).astype(np.float32)
        # v bias broadcast tile [128 rows (tokens), 128 cols (features)]
        vbb_c = np.ascontiguousarray(np.broadcast_to(
            b_qkv[2 * D + f0:2 * D + f0 + 128][None, :], (128, 128))
        ).astype(bf)
        in_maps.append({
            "xT": xT, "wq": wq_c, "bqc": bqc_c, "vbb": vbb_c, "wfc": wfc,
            "bfcc": bfcc, "hm": hm,
        })
    return in_maps


def _get_nc():
    if "nc" not in _CACHE:
        _CACHE["nc"] = _build()
    return _CACHE["nc"]


def _assemble(results):
    full = np.empty((BT, D), dtype=np.float32)
    for c in range(NCORES):
        o = results[c]["outT"]  # [1024 features, 4*256 tokens]
        for b in range(B):
            full[b * T + c * TOK: b * T + (c + 1) * TOK, :] = \
                o[:, b * TOK:(b + 1) * TOK].T
    return full.reshape(B, T, D)


def kernel(x, W_qkv, b_qkv, W_fc, b_fc):
    nc = _get_nc()
    in_maps = _host_inputs(x, W_qkv, b_qkv, W_fc, b_fc)
    res = run_bass_kernel_spmd(nc, in_maps, list(range(NCORES)))
    return _assemble(res.results)


# revision 13
# speedup vs baseline: 1.2140x; 1.0069x over previous
"""Trainium2 Bass kernel for causal multi-head attention (B=4, T=2048, D=1024, H=16).

Sharding: tensor-parallel over heads for QKV+attention (each of 8 cores owns
2 heads over all tokens), then AllToAll re-shards from head-sharded to
token-sharded so each core computes the final FC over the full feature dim
for its 256-token slice of each batch.

All matmuls run in bf16 (fp32 streams at half PE rate; bf16 at full), with
fp32 PSUM accumulation. Scores are computed transposed (S^T = K Q^T, two
heads packed in PE quadrants via tile_position) so softmax normalization
lands on the PV matmul's free dim; the denominator comes from a ones column
augmented into V. Normalization is deferred out of the inner loop (the
per-chunk broadcast chain caused PE bubbles + HAM re-throttling). The
denominator reciprocal is computed as exp(-ln(x)) on the scalar engine: Ln
and Exp share one activation table set (natural_log_exp_and_others), so the
scalar engine never reloads tables mid-kernel (the old Reciprocal function
lives in its own set and caused 32 x 1.3us ACT_TABLE_LOAD stalls on the
exp critical path). QKV/FC biases are added on the DVE during the
PSUM->SBUF copy (tensor_scalar with a per-partition [128,1] bias AP, or a
precomputed broadcast tile for V whose bias varies along the free dim) --
the old ones-vector bias matmuls burned ~27us of PE streaming+LDWEIGHTS.
The broadcast of 1/denom across partitions uses a selector matmul
(partition-shifted DVE/DMA ops misbehave; SBUF-side DMA access patterns
must keep the partition dim outermost). Mask-multiplies and normalize
multiplies run on the otherwise-idle GPSIMD engine (it cannot read PSUM);
diagonal-tile exp+mask handle both heads in one instruction via strided 3D
APs (mask tile duplicated per head). Normalization runs per query-chunk
(one chunk behind attention) and stages its praw slice into the AllToAll
DRAM buffer immediately, so the collective trigger's DMA wait is nearly
zero and the next batch's gpsimd mask-muls are not stalled behind it.
One AllToAll per batch (smaller per-half collectives measured ~3x slower
per byte), QKV of batch b+1 / FC of batch b-1 interleave into batch b's
attention to keep the PE fed while the scalar engine runs exp, and FC of
batch 2 is deferred into the tail to overlap the final AllToAll.
"""
import sys

for _p in ("/opt/trn_rl_repo",):
    if _p not in sys.path:
        sys.path.insert(0, _p)

import numpy as np

import concourse.bass as bass
import concourse.mybir as mybir
import concourse.tile as tile
from concourse import bacc
from concourse.bass_utils import run_bass_kernel_spmd

f32 = mybir.dt.float32
bf16 = mybir.dt.bfloat16
EXP = mybir.ActivationFunctionType.Exp
LN = mybir.ActivationFunctionType.Ln

B, T, D, H, HD = 4, 2048, 1024, 16, 64
NCORES = 8
HPC = H // NCORES          # heads per core = 2
BT = B * T                 # 8192
CH = 512                   # token chunk (q chunk / projection chunk)
NCH_B = T // CH            # 4 projection chunks per batch
QC = T // CH               # 4 query chunks per batch
NKV_B = T // 128           # 16 kv tiles of 128 per batch
TOK = 256                  # tokens per core per batch (after AllToAll)
SCALE = 1.0 / 8.0          # 1/sqrt(HD)

_CACHE = {}


class _Bacc(bacc.Bacc):
    """Bacc whose activation-table pass resolves Exp AND Ln to the combined
    natural_log_exp_and_others set (canonical id 6) instead of their
    individual default sets, so alternating Exp/Ln activations trigger one
    ACT_TABLE_LOAD total instead of one per transition (1.3us each, on the
    softmax critical path)."""

    def insert_act_table_loads(self):
        has_activation = any(
            isinstance(i, mybir.InstActivation)
            for b in self.main_func.blocks
            for i in b.instructions
        )
        if not has_activation:
            return
        from concourse.hw_specs import get_activation_tables
        tables = []
        for name, funcs in get_activation_tables(self.m.arch).items():
            funcs = set(funcs)
            if name == "exp_and_others":
                funcs.discard(mybir.ActivationFunctionType.Exp)
            if name == "natural_log":
                funcs.discard(mybir.ActivationFunctionType.Ln)
            tables.append((name, funcs))
        bacc._bass_rust.insert_act_table_loads(self, tables)


def _build(no_collective=False):
    nc = _Bacc("TRN2", target_bir_lowering=False, debug=False,
               num_devices=NCORES)

    xT = nc.dram_tensor("xT", [D, BT], bf16, kind="ExternalInput").ap()
    wq_d = nc.dram_tensor("wq", [128, 8 * 384], bf16, kind="ExternalInput").ap()
    bqc_d = nc.dram_tensor("bqc", [128, 2], f32, kind="ExternalInput").ap()
    vbb_d = nc.dram_tensor("vbb", [128, 128], bf16, kind="ExternalInput").ap()
    wfc_d = nc.dram_tensor("wfc", [128, 64 * 128], bf16,
                           kind="ExternalInput").ap()
    bfcc_d = nc.dram_tensor("bfcc", [128, 8], f32, kind="ExternalInput").ap()
    hm_d = nc.dram_tensor("hm", [128, 2 * 896], bf16, kind="ExternalInput").ap()
    outT = nc.dram_tensor("outT", [D, B * TOK], f32,
                          kind="ExternalOutput").ap()

    with tile.TileContext(nc) as tc:
        with tc.tile_pool(name="const", bufs=1) as cst, \
             tc.tile_pool(name="dram", bufs=1, space="DRAM") as dpool, \
             tc.tile_pool(name="work", bufs=1) as wk, \
             tc.tile_pool(name="ps", bufs=1, space="PSUM") as ps:

            # ---- weights needed first (QKV of batch 0) ----
            wq = cst.tile([128, 8 * 384], bf16)
            nc.sync.dma_start(wq[:], wq_d[:])
            bqc = cst.tile([128, 2], f32)
            nc.sync.dma_start(bqc[:], bqc_d[:])
            vbb = cst.tile([128, 128], bf16)
            nc.sync.dma_start(vbb[:], vbb_d[:])
            onesb = cst.tile([1, CH], bf16)
            nc.gpsimd.memset(onesb[:], 1.0)
            # selector for reciprocal broadcast: row 64 = 1, rest 0
            zl = cst.tile([65, 64], bf16)
            nc.gpsimd.memset(zl[:], 0.0)
            nc.gpsimd.memset(zl[64:65, :], 1.0)

            # ---- deferred constants (needed later; don't block first mm) ----
            def _late_consts():
                hm = cst.tile([128, 2 * 896], bf16)
                nc.sync.dma_start(hm[:], hm_d[:])
                wfc = cst.tile([128, 64 * 128], bf16)
                nc.sync.dma_start(wfc[:], wfc_d[:])
                bfcc = cst.tile([128, 8], f32)
                nc.sync.dma_start(bfcc[:], bfcc_d[:])
                # per-head reciprocal staging: row 64 = recip, rows 0-63
                # zeroed once (garbage would poison the selector matmul)
                rc = []
                for h in range(HPC):
                    t = cst.tile([65, T], bf16, name=f"rc{h}")
                    nc.gpsimd.memset(t[0:64, :], 0.0)
                    rc.append(t)
                return hm, wfc, bfcc, rc

            # ---- per-batch state (double buffered across the pipeline) ----
            def alloc_batch(b):
                qt = wk.tile([128, T], bf16, tag="qt", bufs=2, name=f"qt{b}")
                kt = wk.tile([128, T], bf16, tag="kt", bufs=2, name=f"kt{b}")
                vsb = wk.tile([128, NKV_B * 130], bf16, tag="vsb", bufs=2,
                              name=f"vsb{b}")
                v3 = vsb.rearrange("p (t c) -> p t c", c=130)
                nc.gpsimd.memset(v3[:, :, 64:65], 1.0)
                nc.gpsimd.memset(v3[:, :, 129:130], 1.0)
                praw = [wk.tile([64, T], bf16, tag=f"praw{h}", bufs=2,
                                name=f"praw{h}_{b}") for h in range(HPC)]
                return qt, kt, vsb, praw

            def qkv_dma(b, ch):
                """Prefetch one 512-token x chunk."""
                c0 = b * T + ch * CH
                xt = wk.tile([128, 8 * CH], bf16, tag="xt", bufs=6,
                             name=f"xt{b}_{ch}")
                xt3 = xt.rearrange("p (d c) -> p d c", d=8)
                xs3 = xT[:, c0:c0 + CH].rearrange("(d p) c -> p d c", p=128)
                nc.sync.dma_start(xt3[:], xs3)
                return xt

            def qkv_units(b, ch, xt, st):
                """Chunk projection as self-contained PE units (aux PSUM).

                Each unit allocates its own short-lived [128,512] aux tile so
                units can interleave with attention without pinning the
                st-tag rotation."""
                qt, kt, vsb = st[0], st[1], st[2]
                cs = ch * CH
                xt3 = xt.rearrange("p (d c) -> p d c", d=8)

                def q_unit(tgt, wo, bcol):
                    def emit():
                        pq = ps.tile([128, CH], f32, tag="aux", bufs=2,
                                     name=f"pq{b}_{ch}_{wo}")
                        for d in range(8):
                            nc.tensor.matmul(pq[:],
                                             wq[:, d * 384 + wo:d * 384 + wo + 128],
                                             xt[:, d * CH:(d + 1) * CH],
                                             start=(d == 0), stop=(d == 7))
                        nc.vector.tensor_scalar_add(
                            tgt[:, cs:cs + CH], pq[:], bqc[:, bcol:bcol + 1])
                    return emit

                def v_unit(sb):
                    def emit():
                        kvt = ch * 4 + sb
                        pvv = ps.tile([128, CH], f32, tag="aux", bufs=2,
                                      name=f"pvv{b}_{ch}_{sb}")
                        for d in range(8):
                            nc.tensor.matmul(
                                pvv[:, 0:128],
                                xt3[:, d, sb * 128:(sb + 1) * 128],
                                wq[:, d * 384 + 256:d * 384 + 384],
                                start=(d == 0), stop=(d == 7))
                        base = kvt * 130
                        nc.vector.tensor_add(vsb[:, base:base + 64],
                                             pvv[:, 0:64], vbb[:, 0:64])
                        nc.vector.tensor_add(vsb[:, base + 65:base + 129],
                                             pvv[:, 64:128], vbb[:, 64:128])
                    return emit

                return [q_unit(qt, 0, 0), q_unit(kt, 128, 1),
                        v_unit(0), v_unit(1), v_unit(2), v_unit(3)]

            def attn_qc(b, qc, st, hm, rc, drain):
                """Causal attention for query chunk qc of batch b."""
                qt, kt, vsb, praw = st
                nkv = 4 * (qc + 1)
                pv = [ps.tile([65, CH], f32, tag=f"pv{h}", bufs=1,
                              name=f"pv{h}_{b}_{qc}") for h in range(HPC)]
                hm3 = hm.rearrange("p (h c) -> p h c", h=2)
                for ki in range(nkv):
                    diag = ki - 4 * qc
                    # on diagonal tiles, queries below the block are fully
                    # masked: shrink the moving dim to the causal range
                    off = 128 * diag if diag > 0 else 0
                    m = CH - off
                    stt = ps.tile([128, 2 * CH], f32, tag="st", bufs=2,
                                  name=f"s_{b}_{qc}_{ki}")
                    pt = wk.tile([128, 2 * CH], bf16, tag="pt", bufs=3,
                                 name=f"p_{b}_{qc}_{ki}")
                    for h in range(HPC):
                        nc.tensor.matmul(
                            stt[:, h * CH + off:(h + 1) * CH],
                            kt[64 * h:64 * h + 64, ki * 128:(ki + 1) * 128],
                            qt[64 * h:64 * h + 64,
                               qc * CH + off:(qc + 1) * CH],
                            start=True, stop=True,
                            tile_position=(64 * h, 0))
                    # fill the scores->exp->PV latency with independent PE
                    # work: the unit lands between S(ki) and PV(ki) in the
                    # in-order PE queue, so PV no longer stalls on exp
                    drain(1)
                    if diag > 0:
                        st3 = stt.rearrange("p (h c) -> p h c", h=2)
                        pt3 = pt.rearrange("p (h c) -> p h c", h=2)
                        nc.scalar.activation(pt3[:, :, off:CH],
                                             st3[:, :, off:CH],
                                             EXP, scale=SCALE)
                    else:
                        nc.scalar.activation(pt[:], stt[:], EXP, scale=SCALE)
                    if diag >= 0:
                        # only columns [off, off+128) of a diagonal tile are
                        # actually triangular-masked (the rest are fully
                        # valid), so multiply just that 128-wide strip --
                        # 4x less gpsimd work on the exp->PV critical path
                        pt3 = pt.rearrange("p (h c) -> p h c", h=2)
                        nc.gpsimd.tensor_mul(pt3[:, :, off:off + 128],
                                             pt3[:, :, off:off + 128],
                                             hm3[:, :, 384:512])
                    for h in range(HPC):
                        vb = ki * 130 + 65 * h
                        if diag >= 0 and m > 128:
                            # split PV: the clean columns don't need to wait
                            # for the mask multiply on the masked strip
                            nc.tensor.matmul(
                                pv[h][0:65, off:off + 128],
                                vsb[:, vb:vb + 65],
                                pt[:, h * CH + off:h * CH + off + 128],
                                start=(ki == 0), stop=(ki == nkv - 1))
                            nc.tensor.matmul(
                                pv[h][0:65, off + 128:CH],
                                vsb[:, vb:vb + 65],
                                pt[:, h * CH + off + 128:(h + 1) * CH],
                                start=(ki == 0), stop=(ki == nkv - 1))
                        else:
                            nc.tensor.matmul(
                                pv[h][0:65, off:CH],
                                vsb[:, vb:vb + 65],
                                pt[:, h * CH + off:(h + 1) * CH],
                                start=(ki == 0),
                                stop=(ki == nkv - 1))
                # stash raw PV + 1/denominator = exp(-ln(denom)); Ln+Exp
                # share one table set so no ACT_TABLE_LOAD is triggered.
                # ln intermediate kept f32 (bf16 would cost ~2% in exp).
                for h in range(HPC):
                    nc.vector.tensor_copy(praw[h][:, qc * CH:(qc + 1) * CH],
                                          pv[h][0:64, :])
                    lnt = wk.tile([1, CH], f32, tag="lnt", bufs=2,
                                  name=f"lnt{h}_{b}_{qc}")
                    nc.scalar.activation(lnt[:], pv[h][64:65, :], LN)
                    nc.scalar.activation(rc[h][64:65, qc * CH:(qc + 1) * CH],
                                         lnt[:], EXP, scale=-1.0)

            def normalize_qc(b, qc, st, rc, ag_in):
                """praw[:, qc] *= broadcast(1/denom), then stage into the
                AllToAll input so the collective trigger has ~no DMA wait."""
                praw = st[3]
                div = ag_in.rearrange("(d p) c -> p d c", p=128)
                for h in range(HPC):
                    bcq = ps.tile([128, CH], f32, tag="aux", bufs=2,
                                  name=f"bc{h}_{b}_{qc}")
                    nc.tensor.matmul(bcq[0:64, :], zl[:, 0:64],
                                     rc[h][:, qc * CH:(qc + 1) * CH],
                                     start=True, stop=True)
                    rbs = wk.tile([64, CH], bf16, tag="rbs", bufs=2,
                                  name=f"rbs{h}_{b}_{qc}")
                    nc.vector.tensor_copy(rbs[:], bcq[0:64, :])
                    nc.gpsimd.tensor_mul(
                        praw[h][:, qc * CH:(qc + 1) * CH],
                        praw[h][:, qc * CH:(qc + 1) * CH], rbs[:])
                    src = praw[h][:, qc * CH:(qc + 1) * CH].rearrange(
                        "p (d c) -> p d c", c=TOK)
                    nc.sync.dma_start(
                        div[64 * h:64 * h + 64, 2 * qc:2 * qc + 2, :], src)

            def a2a_alloc(b):
                ag_in = dpool.tile([NCORES * 128, TOK], bf16,
                                   name=f"ag_in{b}")
                ag_out = dpool.tile([NCORES * 128, TOK], bf16,
                                    name=f"ag_out{b}")
                return ag_in, ag_out

            def a2a_batch(b, ag_in, ag_out):
                """Re-shard batch b attention output: head- to token-sharded."""
                if no_collective:
                    nc.sync.dma_start(ag_out[:], ag_in[:])
                else:
                    nc.gpsimd.collective_compute(
                        "AllToAll", mybir.AluOpType.bypass,
                        replica_groups=[list(range(NCORES))],
                        ins=[ag_in.opt()], outs=[ag_out.opt()])
                return ag_out

            def fc_units(b, ag_out, wfc, bfcc):
                """Full FC for this core's 256-token slice, as PE units."""
                box = {}

                def load():
                    fci = wk.tile([128, 8 * TOK], bf16, tag="fci", bufs=2,
                                  name=f"fci{b}")
                    fci3 = fci.rearrange("p (d c) -> p d c", d=8)
                    srcv = ag_out.rearrange("(d p) c -> p d c", p=128)
                    nc.sync.dma_start(fci3[:], srcv)
                    box["fci"] = fci
                    box["ost"] = wk.tile([128, 8 * TOK], f32, tag="ost",
                                         bufs=2, name=f"ost{b}")

                def fo_unit(fo):
                    def emit():
                        fci, ost = box["fci"], box["ost"]
                        pfc = ps.tile([128, CH], f32, tag="aux", bufs=2,
                                      name=f"pfc{b}_{fo}")
                        for d in range(8):
                            nc.tensor.matmul(
                                pfc[:, 0:TOK],
                                wfc[:, (fo * 8 + d) * 128:
                                     (fo * 8 + d + 1) * 128],
                                fci[:, d * TOK:(d + 1) * TOK],
                                start=(d == 0), stop=(d == 7))
                        nc.vector.tensor_scalar_add(
                            ost[:, fo * TOK:(fo + 1) * TOK], pfc[:, 0:TOK],
                            bfcc[:, fo:fo + 1])
                    return emit

                def store():
                    dst = outT.rearrange("(f p) c -> p f c", p=128)[
                        :, :, b * TOK:(b + 1) * TOK]
                    osrc = box["ost"].rearrange("p (f c) -> p f c", c=TOK)
                    nc.sync.dma_start(dst, osrc)

                return [load] + [fo_unit(fo) for fo in range(8)] + [store]

            def dummy_unit(tag_i):
                """~0.9us of dependency-free matmuls to keep the HAM warm."""
                def emit():
                    pdm = ps.tile([128, CH], f32, tag="aux", bufs=2,
                                  name=f"pdm{tag_i}")
                    for r in range(4):
                        nc.tensor.matmul(pdm[0:1, :], onesb[0:1, 0:1],
                                         onesb[0:1, :],
                                         start=True, stop=True)
                return emit

            # ================= schedule =================
            from collections import deque
            filler = deque()

            def drain(n):
                for _ in range(min(n, len(filler))):
                    filler.popleft()()

            def drain_all():
                while filler:
                    filler.popleft()()

            states = [None] * B
            states[0] = alloc_batch(0)
            xts = {(0, 0): qkv_dma(0, 0)}
            hm, wfc, bfcc, rc = _late_consts()
            # warm up the collective stream so the first real AllToAll does
            # not absorb the cross-core barrier + algorithm setup (~20us)
            if not no_collective:
                w_in = dpool.tile([NCORES, 16], bf16, name="warm_in")
                w_out = dpool.tile([NCORES, 16], bf16, name="warm_out")
                wsb = cst.tile([8, 16], bf16, name="warm_sb")
                nc.gpsimd.memset(wsb[:], 0.0)
                nc.sync.dma_start(w_in[:], wsb[:])
                nc.gpsimd.collective_compute(
                    "AllToAll", mybir.AluOpType.bypass,
                    replica_groups=[list(range(NCORES))],
                    ins=[w_in.opt()], outs=[w_out.opt()])
            # preloop: only chunks 0/1 of batch 0 run serially; its chunks
            # 2/3 become filler inside batch 0's own qc0/qc1
            xts[(0, 1)] = qkv_dma(0, 1)
            for ch in (0, 1):
                for u in qkv_units(0, ch, xts.pop((0, ch)), states[0]):
                    u()
            xts[(0, 2)] = qkv_dma(0, 2)
            xts[(0, 3)] = qkv_dma(0, 3)

            ags = [a2a_alloc(b) for b in range(B)]
            ndum = 0
            # QKV of batch X is staggered: chunks 0/1 run as filler during
            # batch X-1 (qc2/qc3), chunks 2/3 during batch X itself
            # (qc0/qc1).  This gives EVERY batch -- including the last --
            # ~10us of real PE filler in its first half, where previously
            # batch 3 had nothing and the PE micro-idled between S and PV
            # long enough for the HAM to hold the clock at K=4/8 for the
            # whole final quarter of the kernel.  Every chunk's x DMA is
            # issued >= 2 qc slots before its units drain, so the first
            # projection matmul of a batch never waits on HBM.
            # Normalization lags ONE slot globally (not per batch): batch
            # b's last chunk is normalized and its AllToAll triggered only
            # AFTER batch b+1's first attention chunk is emitted, so the
            # scalar engine's exp stream never drains at a batch boundary
            # (previously ~8.5us of scalar idle per boundary while the PE
            # finished the PV tail + normalize at half clock).
            for b in range(B):
                for qc in range(QC):
                    if b > 0 and qc == 0:
                        drain_all()
                    if qc in (0, 1):
                        filler.extend(qkv_units(b, qc + 2,
                                                xts.pop((b, qc + 2)),
                                                states[b]))
                        if b + 1 < B:
                            if qc == 0:
                                states[b + 1] = alloc_batch(b + 1)
                            xts[(b + 1, qc)] = qkv_dma(b + 1, qc)
                    else:
                        if b + 1 < B:
                            filler.extend(qkv_units(b + 1, qc - 2,
                                                    xts.pop((b + 1, qc - 2)),
                                                    states[b + 1]))
                            xts[(b + 1, qc)] = qkv_dma(b + 1, qc)
                    if qc == 2 and b in (1, 2):
                        filler.extend(fc_units(b - 1, ags[b - 1][1],
                                               wfc, bfcc))
                    if b == B - 1 and qc >= 2:
                        # no next-batch QKV to interleave: keep PE warm
                        for _ in range(6):
                            ndum += 1
                            filler.append(dummy_unit(ndum))
                    attn_qc(b, qc, states[b], hm, rc, drain)
                    if b > 0 and qc == 0:
                        normalize_qc(b - 1, QC - 1, states[b - 1], rc,
                                     ags[b - 1][0])
                        a2a_batch(b - 1, *ags[b - 1])
                    if qc >= 1:
                        normalize_qc(b, qc - 1, states[b], rc, ags[b][0])
                if b == B - 1:
                    drain_all()
                    normalize_qc(b, QC - 1, states[b], rc, ags[b][0])
                    a2a_batch(b, *ags[b])
            # tail: FC(2) was deferred here so real PE work (instead of
            # dummies) covers the AllToAll(3) completion window
            for _ in range(4):
                ndum += 1
                dummy_unit(ndum)()
            for u in fc_units(B - 2, ags[B - 2][1], wfc, bfcc):
                u()
            for _ in range(4):
                ndum += 1
                dummy_unit(ndum)()
            for u in fc_units(B - 1, ags[B - 1][1], wfc, bfcc):
                u()

    nc.compile()
    return nc


def _host_inputs(x, W_qkv, b_qkv, W_fc, b_fc):
    import ml_dtypes
    bf = ml_dtypes.bfloat16
    x = np.asarray(x, dtype=np.float32)
    W_qkv = np.asarray(W_qkv, dtype=np.float32)
    b_qkv = np.asarray(b_qkv, dtype=np.float32)
    W_fc = np.asarray(W_fc, dtype=np.float32)
    b_fc = np.asarray(b_fc, dtype=np.float32)

    xT = np.ascontiguousarray(x.reshape(BT, D).T).astype(bf)
    hm1 = (np.arange(128)[:, None]
           <= np.arange(896)[None, :] - 384).astype(bf)
    hm = np.ascontiguousarray(np.concatenate([hm1, hm1], axis=1))
    # full FC weights prepacked to SBUF layout [p, (f*8+d)*128 + c]
    wfc = np.ascontiguousarray(
        W_fc.reshape(8, 128, 8, 128).transpose(1, 2, 0, 3).reshape(128, 8192)
    ).astype(bf)
    # FC bias: column fo = features fo*128..(fo+1)*128 (per-partition)
    bfcc = np.ascontiguousarray(b_fc.reshape(8, 128).T).astype(np.float32)
    in_maps = []
    for c in range(NCORES):
        f0 = c * (HPC * HD)  # 128*c
        wqs = np.concatenate(
            [W_qkv[:, p * D + f0: p * D + f0 + 128] for p in range(3)],
            axis=1)  # [1024, 384] = [q|k|v]
        wq_c = np.ascontiguousarray(
            wqs.reshape(8, 128, 384).transpose(1, 0, 2).reshape(128, 3072)
        ).astype(bf)
        # q/k bias as per-partition columns [128, 2]
        bqc_c = np.ascontiguousarray(np.stack(
            [b_qkv[f0:f0 + 128], b_qkv[D + f0:D + f0 + 128]], axis=1)
        ).astype(np.float32)
        # v bias broadcast tile [128 rows (tokens), 128 cols (features)]
        vbb_c = np.ascontiguousarray(np.broadcast_to(
            b_qkv[2 * D + f0:2 * D + f0 + 128][None, :], (128, 128))
        ).astype(bf)
        in_maps.append({
            "xT": xT, "wq": wq_c, "bqc": bqc_c, "vbb": vbb_c, "wfc": wfc,
            "bfcc": bfcc, "hm": hm,
        })
    return in_maps


def _get_nc():
    if "nc" not in _CACHE:
        _CACHE["nc"] = _build()
    return _CACHE["nc"]


def _assemble(results):
    full = np.empty((BT, D), dtype=np.float32)
    for c in range(NCORES):
        o = results[c]["outT"]  # [1024 features, 4*256 tokens]
        for b in range(B):
            full[b * T + c * TOK: b * T + (c + 1) * TOK, :] = \
                o[:, b * TOK:(b + 1) * TOK].T
    return full.reshape(B, T, D)


def kernel(x, W_qkv, b_qkv, W_fc, b_fc):
    nc = _get_nc()
    in_maps = _host_inputs(x, W_qkv, b_qkv, W_fc, b_fc)
    res = run_bass_kernel_spmd(nc, in_maps, list(range(NCORES)))
    return _assemble(res.results)
